# revision 1
# baseline (speedup 1.0000x reference)
"""Multi-head attention (B=2, N=2048, D=1024, H=16) on 8 Trainium2 cores.

Sharding: data-parallel over batch (2) x tensor-parallel over head groups (4).
Core c handles batch c//4, heads 4*(c%4) .. 4*(c%4)+3.

Per-core kernel (matmuls at full PE rate via float32r, P*V in bf16):
  front:   kT = ([Wk;bk]^T @ [x^T;1])   (channels on partitions)
           v  = ([x;1] @ [Wv;bv])       (tokens on partitions, [v|1] blocks)
  per query-tile of 512 (ACT-exp is the pacing engine, ~32us/tile):
           qT slice = ([Wq;bq]^T @ [x^T;1])
           for each key-ptile of 128:
             S^T[:,h,:] = kT_h^T qT_h    (keys on partitions, 4 single-shot
                                          matmuls into the 4 banks of one tile)
             P^T = exp(SCALE * S^T)      (one ACT op over all 4 heads)
             [O^T_h; sums_h] += [v_h|1]^T P^T_h   (per-head chain, own bank)
           O^T_h *= broadcast(1/sums_h)  (DVE recip + gpsimd bcast + DVE mul)
           out[tokens of this tile] = sum_h O^T_h^T @ Wo_h  (K=64 chains)
Host: out[b] = sum of the 4 group partials + b_o.
"""

import sys

sys.path.insert(0, "/opt/trn_rl_repo")

import numpy as np

B, N, D, H = 2, 2048, 1024, 16
SUB = D // H  # 64
GROUPS = 4  # tensor-parallel head groups
NH = H // GROUPS  # 4 local heads per core
CH = NH * SUB  # 256 local channels
NCORES = 8


def build_nc(NT=N, DK=D, DO=D, nh=NH, name="mha"):
    import concourse.mybir as mybir
    from concourse import bacc
    from concourse.tile import TileContext

    f32 = mybir.dt.float32
    f32r = mybir.dt.float32r
    bf16 = mybir.dt.bfloat16
    Exp = mybir.ActivationFunctionType.Exp
    mult = mybir.AluOpType.mult

    sub = 64
    ch = nh * sub
    KT = DK // 128  # contraction ptiles
    CHT = ch // 128  # channel ptiles
    TOKT = NT // 128  # token/key ptiles
    QT = NT // 512  # query tiles
    TPQ = TOKT // QT  # token ptiles emitted per query tile (4)
    scale = sub ** -0.5

    nc = bacc.Bacc(None, name=name)
    xT = nc.dram_tensor("xT", [DK, NT], f32r, kind="ExternalInput")
    wq = nc.dram_tensor("wq", [DK + 1, ch], f32r, kind="ExternalInput")
    wk = nc.dram_tensor("wk", [DK + 1, ch], f32r, kind="ExternalInput")
    wv = nc.dram_tensor("wv", [DK + 1, ch], f32r, kind="ExternalInput")
    wo = nc.dram_tensor("wo", [ch, DO], f32r, kind="ExternalInput")
    ones_in = nc.dram_tensor("ones", [1, 512], f32r, kind="ExternalInput")
    bq = nc.dram_tensor("bq", [ch, 1], f32, kind="ExternalInput")
    bk = nc.dram_tensor("bk", [ch, 1], f32, kind="ExternalInput")
    out = nc.dram_tensor("out", [NT, DO], f32, kind="ExternalOutput")

    with TileContext(nc) as tc:
        with tc.tile_pool(name="persist", bufs=1) as pp:
            ones = pp.tile([1, 512], f32r)
            qT_sb = pp.tile([128, CHT, NT], f32r)
            kT_sb = pp.tile([128, CHT, NT], f32r)
            v_sb = pp.tile([128, TOKT, ch], bf16)
            ones_c = pp.tile([128, 1], bf16)
            oT_sb = pp.tile([128, CHT, NT], f32r)
            wo_sb = pp.tile([128, CHT, DO], f32r)
            nc.sync.dma_start(ones[:], ones_in[:])
            ones_f = pp.tile([128, 1], f32)
            nc.vector.memset(ones_f[:], 1.0)
            nc.vector.tensor_copy(ones_c[:], ones_f[:])
            zeros_c = pp.tile([128, 128], bf16)
            zeros_f = pp.tile([128, 128], f32)
            nc.vector.memset(zeros_f[:], 0.0)
            nc.vector.tensor_copy(zeros_c[:], zeros_f[:])
            bqk_sb = pp.tile([128, 2, CHT], f32)
            for i, bsrc in enumerate((bq, bk)):
                for ct in range(CHT):
                    nc.sync.dma_start(
                        bqk_sb[:, i, ct : ct + 1], bsrc[ct * 128 : (ct + 1) * 128, :]
                    )
            for ct in range(CHT):
                nc.sync.dma_start(wo_sb[:, ct, :], wo[ct * 128 : (ct + 1) * 128, :])

            with tc.tile_pool(name="xp", bufs=1) as xp, \
                 tc.tile_pool(name="wp", bufs=1) as wp, \
                 tc.tile_pool(name="stp", bufs=2, space="PSUM") as stp, \
                 tc.tile_pool(name="acc", bufs=4, space="PSUM") as acc, \
                 tc.tile_pool(name="ptp", bufs=8) as ptp, \
                 tc.tile_pool(name="nrm", bufs=4) as nrm, \
                 tc.tile_pool(name="osg", bufs=4) as osg:
                xt = xp.tile([128, KT, NT], f32r)
                w_sb = {}

                def load_w(nm, dram):
                    wch = ch
                    wt = wp.tile([128, KT, wch], f32r, name=f"{nm}t", tag=f"{nm}t")
                    for kt in range(KT):
                        nc.sync.dma_start(
                            wt[:, kt, :], dram[kt * 128 : (kt + 1) * 128, :]
                        )
                    wb = wp.tile([1, wch], f32r, name=f"{nm}b", tag=f"{nm}b")
                    nc.sync.dma_start(wb[:], dram[DK : DK + 1, :])
                    w_sb[nm] = (wt, wb)

                load_w("wk", wk)
                for kt in range(KT):
                    nc.sync.dma_start(xt[:, kt, :], xT[kt * 128 : (kt + 1) * 128, :])
                load_w("wv", wv)
                load_w("wq", wq)

                def qk_proj(dst, nm, mt, qt, pool=None, tag="acc"):
                    """dst[:, mt, qt*512:+512] = (W^T @ x^T) slice + per-partition bias."""
                    wt, wb = w_sb[nm]
                    ps = (pool or acc).tile([128, 512], f32, name="ps", tag=tag)
                    for kt in range(KT):
                        nc.tensor.matmul(
                            ps[:],
                            lhsT=wt[:, kt, mt * 128 : (mt + 1) * 128],
                            rhs=xt[:, kt, qt * 512 : (qt + 1) * 512],
                            start=(kt == 0),
                            stop=(kt == KT - 1),
                        )
                    nc.vector.tensor_scalar_add(
                        dst[:, mt, qt * 512 : (qt + 1) * 512],
                        ps[:],
                        bqk_sb[:, 0 if nm == "wq" else 1, mt : mt + 1],
                    )

                def v_proj(tt, pool=None, tag="acc"):
                    """v_sb[:, tt, :] = ([x;1] @ [Wv;bv])."""
                    wt, wb = w_sb["wv"]
                    ps = (pool or acc).tile([128, ch], f32, name="psv", tag=tag)
                    for kt in range(KT):
                        nc.tensor.matmul(
                            ps[:],
                            lhsT=xt[:, kt, tt * 128 : (tt + 1) * 128],
                            rhs=wt[:, kt, :],
                            start=(kt == 0),
                            stop=False,
                        )
                    nc.tensor.matmul(
                        ps[:],
                        lhsT=ones[0:1, 0:128],
                        rhs=wb[:],
                        start=False,
                        stop=True,
                    )
                    nc.vector.tensor_copy(v_sb[:, tt, :], ps[:])

                def outproj_piece(tt, nt):
                    ps = acc.tile([128, 512], f32, name="ops", tag="acc")
                    for ct in range(CHT):
                        nc.tensor.matmul(
                            ps[:],
                            lhsT=oT_sb[:, ct, tt * 128 : (tt + 1) * 128],
                            rhs=wo_sb[:, ct, nt * 512 : (nt + 1) * 512],
                            start=(ct == 0),
                            stop=(ct == CHT - 1),
                        )
                    stg = osg.tile([128, 512], f32, name="stg", tag="stg")
                    nc.vector.tensor_copy(stg[:], ps[:])
                    nc.sync.dma_start(
                        out[tt * 128 : (tt + 1) * 128, nt * 512 : (nt + 1) * 512],
                        stg[:],
                    )
                # streamed projections: minimal front, everything else
                # trickles through the spare acc slot under the ACT-paced loop
                from collections import deque

                NVF = 2  # v tiles projected upfront; the rest stream just-in-time
                pending = deque()
                for tt in range(NVF, TOKT):
                    pending.append(("v", tt))
                for qt in range(1, QT):
                    for mt in range(CHT):
                        pending.append(("q", mt, qt))

                def emit(item):
                    kind = item[0]
                    if kind == "v":
                        v_proj(item[1])
                    elif kind == "q":
                        qk_proj(qT_sb, "wq", item[1], item[2])
                    elif kind == "o":
                        outproj_piece(item[1], item[2])

                # minimal front: all of kT, first two v tiles, qT of qt 0.
                # chains alternate between the acc slots and the (idle) stp
                # slots so six are in flight instead of four
                front = [("k", mt, qt) for mt in range(CHT) for qt in range(QT)]
                front += [("vf", tt) for tt in range(NVF)]
                front += [("qf", mt) for mt in range(CHT)]
                for i, item in enumerate(front):
                    pool, tag = (stp, "st") if i % 2 else (acc, "acc")
                    if item[0] == "k":
                        qk_proj(kT_sb, "wk", item[1], item[2], pool=pool, tag=tag)
                    elif item[0] == "vf":
                        v_proj(item[1], pool=pool, tag=tag)
                    else:
                        qk_proj(qT_sb, "wq", item[1], 0, pool=pool, tag=tag)
                for qt in range(QT):
                    ot = [
                        acc.tile([128, 512], f32, name=f"otp{p}", tag="acc")
                        for p in range(nh // 2)
                    ]
                    sm = acc.tile([97, 512], f32, name="sm", tag="acc")
                    for kt2 in range(TOKT):
                        if pending and (pending[0][0] == "v" or kt2 % 2 == 0):
                            emit(pending.popleft())
                        first, last = kt2 == 0, kt2 == TOKT - 1
                        # two half-tiles (2 heads / 2 banks each), double-buffered:
                        # exp of one half pipelines against S-matmuls of the other
                        for half in range(nh // 2):
                            st = stp.tile([128, 2, 512], f32, name="st", tag="st")
                            for hh in range(2):
                                h = 2 * half + hh
                                bp = 64 * hh
                                nc.tensor.matmul(
                                    st[:, hh, :],
                                    lhsT=kT_sb[bp : bp + 64, half, kt2 * 128 : (kt2 + 1) * 128],
                                    rhs=qT_sb[bp : bp + 64, half, qt * 512 : (qt + 1) * 512],
                                    start=True,
                                    stop=True,
                                )
                            pt = ptp.tile([128, 2, 512], bf16, name="pt", tag="pt")
                            nc.scalar.activation(pt[:], st[:], Exp, scale=scale)
                            if first and half == 0:
                                # open the shared-bank has_written groups with
                                # zero matmuls, emitted after the first S/exp so
                                # they don't head-of-line-block the PE stream on
                                # the previous qt's normalize
                                for pp_ in range(nh // 2):
                                    nc.tensor.matmul(
                                        ot[pp_][:], lhsT=zeros_c[:], rhs=v_sb[:, 0:2, :],
                                        start=True, stop=False, skip_group_check=True,
                                    )
                                nc.tensor.matmul(
                                    sm[:], lhsT=zeros_c[:, 0:97], rhs=v_sb[:, 0:2, :],
                                    start=True, stop=False, skip_group_check=True,
                                )
                            for hh in range(2):
                                h = 2 * half + hh
                                nc.tensor.matmul(
                                    ot[half][64 * hh : 64 * hh + 64, :],
                                    lhsT=v_sb[:, kt2, 64 * h : 64 * h + 64],
                                    rhs=pt[:, hh, :],
                                    start=False,
                                    stop=last,
                                    skip_group_check=True,
                                )
                                nc.tensor.matmul(
                                    sm[32 * h : 32 * h + 1, :],
                                    lhsT=ones_c[:],
                                    rhs=pt[:, hh, :],
                                    start=False,
                                    stop=last,
                                    tile_position=(0, 32 * h),
                                    skip_group_check=True,
                                )
                    for h in range(nh):
                        bp = 64 * (h % 2)
                        rcp = nrm.tile([97, 512], f32, name="rcp", tag="rcp")
                        row0 = nrm.tile([1, 512], f32, name="row0", tag="row0")
                        bc = nrm.tile([64, 512], f32, name="bc", tag="bc")
                        nc.vector.reciprocal(rcp[32 * h : 32 * h + 1, :], sm[32 * h : 32 * h + 1, :])
                        # gpsimd broadcast reads physical partition 0: stage there
                        nc.sync.dma_start(row0[:], rcp[32 * h : 32 * h + 1, :])
                        nc.gpsimd.partition_broadcast(bc[:], row0[:], channels=64)
                        nc.vector.tensor_tensor(
                            out=oT_sb[bp : bp + 64, h // 2, qt * 512 : (qt + 1) * 512],
                            in0=ot[h // 2][bp : bp + 64, :],
                            in1=bc[:],
                            op=mult,
                        )
                    for tt in range(qt * TPQ, min((qt + 1) * TPQ, TOKT)):
                        for nt in range(DO // 512):
                            pending.append(("o", tt, nt))
                while pending:
                    emit(pending.popleft())
    nc.finalize()
    return nc


def make_in_maps(x, W_qkv, b_qkv, W_o):
    """Shard full inputs into per-core input maps (core c: batch c//4, group c%4)."""
    x = np.asarray(x, dtype=np.float32)
    W_qkv = np.asarray(W_qkv, dtype=np.float32)
    b_qkv = np.asarray(b_qkv, dtype=np.float32)
    W_o = np.asarray(W_o, dtype=np.float32)
    in_maps = []
    for c in range(NCORES):
        b, g = divmod(c, GROUPS)
        cols = slice(CH * g, CH * (g + 1))
        m = {
            "xT": np.ascontiguousarray(x[b].T),
            "wq": np.ascontiguousarray(
                np.concatenate([W_qkv[:, 0 * D : 1 * D][:, cols], b_qkv[0 * D : 1 * D][cols][None, :]], 0)
            ),
            "wk": np.ascontiguousarray(
                np.concatenate([W_qkv[:, 1 * D : 2 * D][:, cols], b_qkv[1 * D : 2 * D][cols][None, :]], 0)
            ),
            "wv": np.ascontiguousarray(
                np.concatenate([W_qkv[:, 2 * D : 3 * D][:, cols], b_qkv[2 * D : 3 * D][cols][None, :]], 0)
            ),
            "wo": np.ascontiguousarray(W_o[cols, :]),
            "ones": np.ones((1, 512), dtype=np.float32),
            "bq": np.ascontiguousarray(b_qkv[0 * D : 1 * D][cols][:, None]),
            "bk": np.ascontiguousarray(b_qkv[1 * D : 2 * D][cols][:, None]),
        }
        in_maps.append(m)
    return in_maps


_NC = None


def get_nc():
    global _NC
    if _NC is None:
        _NC = build_nc()
    return _NC


def kernel(x, W_qkv, b_qkv, W_o, b_o):
    from concourse import bass_utils

    b_o = np.asarray(b_o, dtype=np.float32)
    in_maps = make_in_maps(x, W_qkv, b_qkv, W_o)
    res = bass_utils.run_bass_kernel_spmd(get_nc(), in_maps, core_ids=list(range(NCORES)))
    out = np.empty((B, N, D), dtype=np.float32)
    for b in range(B):
        acc = res.results[4 * b]["out"].copy()
        for g in range(1, GROUPS):
            acc += res.results[4 * b + g]["out"]
        out[b] = acc + b_o
    return out



# revision 3
# speedup vs baseline: 1.4880x; 1.4880x over previous
"""Multi-head attention (B=2, N=2048, D=1024, H=16) on 8 Trainium2 cores.

Sharding: data-parallel over batch (2) x tensor-parallel over head groups (4).
Core c handles batch c//4, heads 4*(c%4) .. 4*(c%4)+3.

Per-core kernel, all matmul operands bf16 (rate 1.0 cycles/row at any width):
  front:   kT/qT = (W^T @ x^T) + bias   (channels on partitions)
           v     = ([x;1] @ [Wv;bv])    (tokens on partitions)
  per query-tile of 256 (ACT-exp paces at ~1.04us/key-ptile):
    per key-ptile kt2 (128 keys):
      S^T[:,h,:] = kT_h^T qT_h          (keys on partitions, 4 matmuls)
      P^T = exp(SCALE * S^T)            (one ACT op over all 4 heads)
      O[q,s,h]  += P^T_slice^T @ v_h    (64-col streams, queries on partitions)
      sums[q,h] += P^T_slice^T @ 1      (1-col streams, ~free)
    O /= sums (DVE per-partition scalars) -> transpose back (PE, identity)
    out[tokens] = sum_ct oT_ct^T @ Wo_ct  (chains of 2, K=128 each)
Host: out[b] = sum of the 4 group partials + b_o.
"""

import sys

sys.path.insert(0, "/opt/trn_rl_repo")

import numpy as np

B, N, D, H = 2, 2048, 1024, 16
SUB = D // H  # 64
GROUPS = 4  # tensor-parallel head groups
NH = H // GROUPS  # 4 local heads per core
CH = NH * SUB  # 256 local channels
NCORES = 8
QB = 256  # query tile
NQT = N // QB  # 8
KT = D // 128  # contraction ptiles
TOKT = N // 128  # token/key ptiles
SCALE = SUB ** -0.5


def build_nc(name="mha"):
    import concourse.mybir as mybir
    from concourse import bacc
    from concourse.tile import TileContext

    f32 = mybir.dt.float32
    bf16 = mybir.dt.bfloat16
    Exp = mybir.ActivationFunctionType.Exp

    nc = bacc.Bacc(None, name=name)
    # host-packed, partition-major layouts (see make_in_maps)
    xh = nc.dram_tensor("xh", [128, KT, N], bf16, kind="ExternalInput")
    wq = nc.dram_tensor("wq", [128, KT, CH], bf16, kind="ExternalInput")
    wk = nc.dram_tensor("wk", [128, KT, CH], bf16, kind="ExternalInput")
    wv = nc.dram_tensor("wv", [128, KT, CH], bf16, kind="ExternalInput")
    wvb = nc.dram_tensor("wvb", [1, CH], bf16, kind="ExternalInput")
    wo = nc.dram_tensor("wo", [128, 2, D], bf16, kind="ExternalInput")
    bqk = nc.dram_tensor("bqk", [128, 4], f32, kind="ExternalInput")
    ones_d = nc.dram_tensor("ones", [128, 128], bf16, kind="ExternalInput")
    ident_d = nc.dram_tensor("ident", [128, 128], bf16, kind="ExternalInput")
    out = nc.dram_tensor("out", [N, D], bf16, kind="ExternalOutput")

    with TileContext(nc) as tc:
        with tc.tile_pool(name="persist", bufs=1) as pp:
            xt = pp.tile([128, KT, N], bf16)
            wq_sb = pp.tile([128, KT, CH], bf16)
            wk_sb = pp.tile([128, KT, CH], bf16)
            wv_sb = pp.tile([128, KT, CH], bf16)
            wvb_sb = pp.tile([1, CH], bf16)
            wo_sb = pp.tile([128, 2, D], bf16)
            qT_sb = pp.tile([128, 2, N], bf16)
            kT_sb = pp.tile([128, 2, N], bf16)
            v_sb = pp.tile([128, TOKT, CH], bf16)
            oT_sb = pp.tile([128, 2, N], bf16)
            bqk_sb = pp.tile([128, 4], f32)
            ones_sb = pp.tile([128, 128], bf16)
            ident_sb = pp.tile([128, 128], bf16)

            # DMA issue order: earliest-needed first. x comes in 4 token
            # quarters so the first projections can start at ~4.5us.
            nc.sync.dma_start(wk_sb[:], wk[:])
            nc.sync.dma_start(xt[:, :, 0:512], xh[:, :, 0:512])
            nc.sync.dma_start(wq_sb[:], wq[:])
            nc.sync.dma_start(wv_sb[:], wv[:])
            nc.sync.dma_start(wvb_sb[:], wvb[:])
            nc.sync.dma_start(bqk_sb[:], bqk[:])
            nc.sync.dma_start(ones_sb[:], ones_d[:])
            nc.sync.dma_start(ident_sb[:], ident_d[:])
            nc.sync.dma_start(xt[:, :, 512:1024], xh[:, :, 512:1024])
            nc.sync.dma_start(wo_sb[:], wo[:])
            nc.sync.dma_start(xt[:, :, 1024:1536], xh[:, :, 1024:1536])
            nc.sync.dma_start(xt[:, :, 1536:2048], xh[:, :, 1536:2048])

            with tc.tile_pool(name="stp", bufs=2, space="PSUM") as stp, \
                 tc.tile_pool(name="op_", bufs=1, space="PSUM") as op_, \
                 tc.tile_pool(name="smp", bufs=1, space="PSUM") as smp, \
                 tc.tile_pool(name="aux", bufs=2, space="PSUM") as aux, \
                 tc.tile_pool(name="ptp", bufs=3) as ptp, \
                 tc.tile_pool(name="osb", bufs=3) as osb, \
                 tc.tile_pool(name="rcpp", bufs=2) as rcpp, \
                 tc.tile_pool(name="stg", bufs=2) as stgp:

                def qk_proj(dst, wt, bcol, mt, s):
                    """dst[:, mt, 256s:+256] = (W^T x^T)[128ch, 256tok] + bias."""
                    ps = aux.tile([128, 512], f32, name="ps", tag="aux")
                    for kt in range(KT):
                        nc.tensor.matmul(
                            ps[:, 0:QB],
                            lhsT=wt[:, kt, mt * 128 : (mt + 1) * 128],
                            rhs=xt[:, kt, s * QB : (s + 1) * QB],
                            start=(kt == 0),
                            stop=(kt == KT - 1),
                        )
                    nc.vector.tensor_scalar_add(
                        dst[:, mt, s * QB : (s + 1) * QB],
                        ps[:, 0:QB],
                        bqk_sb[:, bcol + mt : bcol + mt + 1],
                    )

                def v_proj(tt):
                    """v_sb[:, tt, :] = ([x;1] @ [Wv;bv])[128tok, 256ch]."""
                    ps = aux.tile([128, 512], f32, name="psv", tag="aux")
                    for kt in range(KT):
                        nc.tensor.matmul(
                            ps[:, 0:CH],
                            lhsT=xt[:, kt, tt * 128 : (tt + 1) * 128],
                            rhs=wv_sb[:, kt, :],
                            start=(kt == 0),
                            stop=False,
                        )
                    nc.tensor.matmul(
                        ps[:, 0:CH],
                        lhsT=ones_sb[0:1, :],
                        rhs=wvb_sb[:],
                        start=False,
                        stop=True,
                    )
                    nc.vector.tensor_copy(v_sb[:, tt, :], ps[:, 0:CH])

                def transpose_pair(osb_t, qt, s):
                    """oT_sb[:, blk, qt*256+128s:+128] = osb_t[:, blk, :]^T."""
                    tr = aux.tile([128, 2, 128], bf16, name="tr", tag="aux")
                    for blk in range(2):
                        nc.tensor.transpose(
                            tr[:, blk, :], osb_t[:, blk, :], ident_sb[:]
                        )
                    for blk in range(2):
                        nc.vector.tensor_copy(
                            oT_sb[:, blk, qt * QB + 128 * s : qt * QB + 128 * (s + 1)],
                            tr[:, blk, :],
                        )

                def outproj(tt):
                    """out[tt*128:+128, :] = sum_ct oT_ct^T @ Wo_ct."""
                    stg = stgp.tile([128, D], bf16, name="stg", tag="stg")
                    for nt in range(2):
                        ps = aux.tile([128, 512], f32, name="ops", tag="aux")
                        for ct in range(2):
                            nc.tensor.matmul(
                                ps[:],
                                lhsT=oT_sb[:, ct, tt * 128 : (tt + 1) * 128],
                                rhs=wo_sb[:, ct, nt * 512 : (nt + 1) * 512],
                                start=(ct == 0),
                                stop=(ct == 1),
                            )
                        nc.vector.tensor_copy(stg[:, nt * 512 : (nt + 1) * 512], ps[:])
                    nc.sync.dma_start(out[tt * 128 : (tt + 1) * 128, :], stg[:])

                def emit(item):
                    kind = item[0]
                    if kind == "q":
                        qk_proj(qT_sb, wq_sb, 0, item[1], item[2])
                    elif kind == "k":
                        qk_proj(kT_sb, wk_sb, 2, item[1], item[2])
                    elif kind == "v":
                        v_proj(item[1])
                    elif kind == "tr":
                        transpose_pair(item[1], item[2], item[3])
                    elif kind == "o":
                        outproj(item[1])

                from collections import deque

                pending = deque()
                # qt0 backlog, ordered by first-use time (k(s) at kt2=2s,
                # v(tt) at kt2=tt, q(qt1) before qt1).
                for it in [
                    ("v", 2), ("v", 3), ("k", 0, 1), ("k", 1, 1),
                    ("v", 4), ("k", 0, 2), ("k", 1, 2), ("v", 5),
                    ("k", 0, 3), ("k", 1, 3), ("v", 6), ("v", 7),
                    ("k", 0, 4), ("k", 1, 4), ("v", 8), ("v", 9),
                    ("k", 0, 5), ("k", 1, 5), ("v", 10), ("v", 11),
                    ("k", 0, 6), ("k", 1, 6), ("v", 12),
                    ("k", 0, 7), ("k", 1, 7), ("v", 13), ("v", 14), ("v", 15),
                    ("q", 0, 1), ("q", 1, 1),
                ]:
                    pending.append(it)

                # front: projections for tokens 0-255 (x quarter 0 only)
                for mt in range(2):
                    qk_proj(kT_sb, wk_sb, 2, mt, 0)
                for mt in range(2):
                    qk_proj(qT_sb, wq_sb, 0, mt, 0)
                v_proj(0)
                v_proj(1)

                def s_exp(qt, kt2):
                    """S^T + exp for (qt, kt2); returns the pt tile."""
                    st = stp.tile([128, NH, QB], f32, name="st", tag="st")
                    for h in range(NH):
                        a = h % 2
                        nc.tensor.matmul(
                            st[:, h, :],
                            lhsT=kT_sb[
                                64 * a : 64 * a + 64,
                                h // 2,
                                kt2 * 128 : (kt2 + 1) * 128,
                            ],
                            rhs=qT_sb[
                                64 * a : 64 * a + 64,
                                h // 2,
                                qt * QB : (qt + 1) * QB,
                            ],
                            start=(h % 2 == 0),  # bank opener per 2 heads
                            stop=True,
                            skip_group_check=True,
                        )
                    pt = ptp.tile([128, NH, QB], bf16, name="pt", tag="pt")
                    nc.scalar.activation(pt[:], st[:], Exp, scale=SCALE)
                    return pt

                pt_next = s_exp(0, 0)
                for qt in range(NQT):
                    o_ps = op_.tile([128, 2, NH, SUB], f32, name="o_ps", tag="o")
                    sm_ps = smp.tile([128, 512], f32, name="sm_ps", tag="sm")
                    for kt2 in range(TOKT):
                        first, last = kt2 == 0, kt2 == TOKT - 1
                        pt = pt_next
                        # software pipeline: S/exp of the NEXT tile go ahead of
                        # this tile's PV so the ACT stream never waits on PE
                        if not last:
                            pt_next = s_exp(qt, kt2 + 1)
                        elif qt + 1 < NQT:
                            pt_next = s_exp(qt + 1, 0)
                        for s in range(2):
                            for h in range(NH):
                                nc.tensor.matmul(
                                    o_ps[:, s, h, :],
                                    lhsT=pt[:, h, 128 * s : 128 * (s + 1)],
                                    rhs=v_sb[:, kt2, 64 * h : 64 * h + 64],
                                    start=(first and s == 0 and h == 0),
                                    stop=last,
                                    skip_group_check=True,
                                )
                                nc.tensor.matmul(
                                    sm_ps[:, 4 * s + h : 4 * s + h + 1],
                                    lhsT=pt[:, h, 128 * s : 128 * (s + 1)],
                                    rhs=ones_sb[:, 0:1],
                                    start=(first and s == 0 and h == 0),
                                    stop=last,
                                    skip_group_check=True,
                                )
                        if pending:
                            emit(pending.popleft())
                    rcp = rcpp.tile([128, 8], f32, name="rcp", tag="rcp")
                    nc.vector.reciprocal(rcp[:], sm_ps[:, 0:8])
                    ot = []
                    for s in range(2):
                        osb_t = osb.tile([128, 2, 128], bf16, name="osb", tag="osb")
                        ot.append(osb_t)
                        for h in range(NH):
                            nc.vector.tensor_scalar_mul(
                                osb_t[:, h // 2, 64 * (h % 2) : 64 * (h % 2) + 64],
                                o_ps[:, s, h, :],
                                rcp[:, 4 * s + h : 4 * s + h + 1],
                            )
                    # post-qt work runs during the next qt's ACT windows
                    pending.append(("tr", ot[0], qt, 0))
                    pending.append(("tr", ot[1], qt, 1))
                    pending.append(("o", 2 * qt))
                    pending.append(("o", 2 * qt + 1))
                    if qt + 1 < NQT:
                        pending.append(("q", 0, qt + 1))
                        pending.append(("q", 1, qt + 1))
                while pending:
                    emit(pending.popleft())
    nc.finalize()
    return nc


def make_in_maps(x, W_qkv, b_qkv, W_o):
    """Shard full inputs into per-core input maps (core c: batch c//4, group c%4)."""
    import ml_dtypes

    bf16 = ml_dtypes.bfloat16
    x = np.asarray(x, dtype=np.float32)
    W_qkv = np.asarray(W_qkv, dtype=np.float32)
    b_qkv = np.asarray(b_qkv, dtype=np.float32)
    W_o = np.asarray(W_o, dtype=np.float32)

    def pack_w(w):  # [1024, CH] -> [128, KT, CH] partition-major
        return np.ascontiguousarray(
            w.reshape(KT, 128, CH).transpose(1, 0, 2).astype(bf16)
        )

    in_maps = []
    for c in range(NCORES):
        b, g = divmod(c, GROUPS)
        cols = slice(CH * g, CH * (g + 1))
        bq = b_qkv[0 * D : 1 * D][cols]
        bk = b_qkv[1 * D : 2 * D][cols]
        bqk = np.stack(
            [bq[0:128], bq[128:256], bk[0:128], bk[128:256]], axis=1
        ).astype(np.float32)
        xh = (
            x[b].T.reshape(KT, 128, N).transpose(1, 0, 2).astype(bf16)
        )  # [128, KT, N]
        m = {
            "xh": np.ascontiguousarray(xh),
            "wq": pack_w(W_qkv[:, 0 * D : 1 * D][:, cols]),
            "wk": pack_w(W_qkv[:, 1 * D : 2 * D][:, cols]),
            "wv": pack_w(W_qkv[:, 2 * D : 3 * D][:, cols]),
            "wvb": np.ascontiguousarray(
                b_qkv[2 * D : 3 * D][cols][None, :].astype(bf16)
            ),
            "wo": np.ascontiguousarray(
                W_o[cols, :].reshape(2, 128, D).transpose(1, 0, 2).astype(bf16)
            ),
            "bqk": np.ascontiguousarray(bqk),
            "ones": np.ones((128, 128), dtype=bf16),
            "ident": np.eye(128, dtype=bf16),
        }
        in_maps.append(m)
    return in_maps


_NC = None


def get_nc():
    global _NC
    if _NC is None:
        _NC = build_nc()
    return _NC


def kernel(x, W_qkv, b_qkv, W_o, b_o):
    from concourse import bass_utils

    b_o = np.asarray(b_o, dtype=np.float32)
    in_maps = make_in_maps(x, W_qkv, b_qkv, W_o)
    res = bass_utils.run_bass_kernel_spmd(get_nc(), in_maps, core_ids=list(range(NCORES)))
    out = np.empty((B, N, D), dtype=np.float32)
    for b in range(B):
        acc = res.results[4 * b]["out"].astype(np.float32)
        for g in range(1, GROUPS):
            acc += res.results[4 * b + g]["out"].astype(np.float32)
        out[b] = acc + b_o
    return out


# revision 8
# speedup vs baseline: 1.5558x; 1.0455x over previous
"""Multi-head attention (B=2, N=2048, D=1024, H=16) on 8 Trainium2 cores.

Sharding: data-parallel over batch (2) x tensor-parallel over head groups (4).
Core c handles batch c//4, heads 4*(c%4) .. 4*(c%4)+3.

Per-core kernel, all matmul operands bf16 (rate 1.0 cycles/row at any width):
  front:   kT/qT = (W^T @ x^T) + bias   (channels on partitions)
           v     = ([x;1] @ [Wv;bv])    (tokens on partitions)
  per query-tile of 256 (ACT-exp paces at ~1.04us/key-ptile):
    per key-ptile kt2 (128 keys):
      S^T[:,h,:] = kT_h^T qT_h          (keys on partitions, 4 matmuls)
      P^T = exp(SCALE * S^T)            (one ACT op over all 4 heads)
      O[q,s,h]  += P^T_slice^T @ v_h    (64-col streams, queries on partitions)
      sums[q,h] += P^T_slice^T @ 1      (1-col streams, ~free)
    O /= sums (DVE per-partition scalars) -> transpose back (PE, identity)
    out[tokens] = sum_ct oT_ct^T @ Wo_ct  (chains of 2, K=128 each)
Host: out[b] = sum of the 4 group partials + b_o.
"""

import sys

sys.path.insert(0, "/opt/trn_rl_repo")

import numpy as np

B, N, D, H = 2, 2048, 1024, 16
SUB = D // H  # 64
GROUPS = 4  # tensor-parallel head groups
NH = H // GROUPS  # 4 local heads per core
CH = NH * SUB  # 256 local channels
NCORES = 8
QB = 256  # query tile
NQT = N // QB  # 8
KT = D // 128  # contraction ptiles
TOKT = N // 128  # token/key ptiles
SCALE = SUB ** -0.5


def build_nc(name="mha"):
    import concourse.mybir as mybir
    from concourse import bacc
    from concourse.tile import TileContext

    f32 = mybir.dt.float32
    bf16 = mybir.dt.bfloat16
    Exp = mybir.ActivationFunctionType.Exp

    nc = bacc.Bacc(None, name=name)
    # host-packed, partition-major layouts (see make_in_maps)
    xh = nc.dram_tensor("xh", [128, KT, N], bf16, kind="ExternalInput")
    wq = nc.dram_tensor("wq", [128, KT, CH], bf16, kind="ExternalInput")
    wk = nc.dram_tensor("wk", [128, KT, CH], bf16, kind="ExternalInput")
    wv = nc.dram_tensor("wv", [128, KT, CH], bf16, kind="ExternalInput")
    wvb = nc.dram_tensor("wvb", [1, CH], bf16, kind="ExternalInput")
    wo = nc.dram_tensor("wo", [128, 2, D], bf16, kind="ExternalInput")
    bqk = nc.dram_tensor("bqk", [128, 4], f32, kind="ExternalInput")
    ones_d = nc.dram_tensor("ones", [128, 128], bf16, kind="ExternalInput")
    ident_d = nc.dram_tensor("ident", [128, 128], bf16, kind="ExternalInput")
    out = nc.dram_tensor("out", [N, D], bf16, kind="ExternalOutput")

    with TileContext(nc) as tc:
        with tc.tile_pool(name="persist", bufs=1) as pp:
            xt = pp.tile([128, KT, N], bf16)
            wq_sb = pp.tile([128, KT, CH], bf16)
            wk_sb = pp.tile([128, KT, CH], bf16)
            wv_sb = pp.tile([128, KT, CH], bf16)
            wvb_sb = pp.tile([1, CH], bf16)
            wo_sb = pp.tile([128, 2, D], bf16)
            qT_sb = pp.tile([128, 2, N], bf16)
            kT_sb = pp.tile([128, 2, N], bf16)
            v_sb = pp.tile([128, TOKT, CH], bf16)
            oT_sb = pp.tile([128, 2, N], bf16)
            bqk_sb = pp.tile([128, 4], f32)
            ones_sb = pp.tile([128, 128], bf16)
            ident_sb = pp.tile([128, 128], bf16)

            # DMA issue order: earliest-needed first. x comes in 4 token
            # quarters so the first projections can start at ~4.5us.
            nc.sync.dma_start(wk_sb[:], wk[:])
            nc.sync.dma_start(xt[:, :, 0:256], xh[:, :, 0:256])
            nc.sync.dma_start(wq_sb[:], wq[:])
            nc.sync.dma_start(xt[:, :, 256:512], xh[:, :, 256:512])
            nc.sync.dma_start(wv_sb[:], wv[:])
            nc.sync.dma_start(wvb_sb[:], wvb[:])
            nc.sync.dma_start(bqk_sb[:], bqk[:])
            nc.sync.dma_start(ones_sb[:], ones_d[:])
            nc.sync.dma_start(xt[:, :, 512:768], xh[:, :, 512:768])
            nc.sync.dma_start(ident_sb[:], ident_d[:])
            nc.sync.dma_start(xt[:, :, 768:1024], xh[:, :, 768:1024])
            nc.sync.dma_start(wo_sb[:], wo[:])
            for sl in range(4, 8):
                nc.sync.dma_start(
                    xt[:, :, sl * 256 : (sl + 1) * 256],
                    xh[:, :, sl * 256 : (sl + 1) * 256],
                )

            with tc.tile_pool(name="stp", bufs=2, space="PSUM") as stp, \
                 tc.tile_pool(name="op_", bufs=1, space="PSUM") as op_, \
                 tc.tile_pool(name="smp", bufs=1, space="PSUM") as smp, \
                 tc.tile_pool(name="aux", bufs=2, space="PSUM") as aux, \
                 tc.tile_pool(name="ptp", bufs=8) as ptp, \
                 tc.tile_pool(name="osb", bufs=3) as osb, \
                 tc.tile_pool(name="rcpp", bufs=2) as rcpp, \
                 tc.tile_pool(name="stg", bufs=2) as stgp:

                def qk_proj(dst, wt, bcol, mt, s):
                    """dst[:, mt, 256s:+256] = (W^T x^T)[128ch, 256tok] + bias."""
                    ps = aux.tile([128, 512], f32, name="ps", tag="aux")
                    for kt in range(KT):
                        nc.tensor.matmul(
                            ps[:, 0:QB],
                            lhsT=wt[:, kt, mt * 128 : (mt + 1) * 128],
                            rhs=xt[:, kt, s * QB : (s + 1) * QB],
                            start=(kt == 0),
                            stop=(kt == KT - 1),
                        )
                    nc.vector.tensor_scalar_add(
                        dst[:, mt, s * QB : (s + 1) * QB],
                        ps[:, 0:QB],
                        bqk_sb[:, bcol + mt : bcol + mt + 1],
                    )

                def v_proj(tt):
                    """v_sb[:, tt, :] = ([x;1] @ [Wv;bv])[128tok, 256ch]."""
                    ps = aux.tile([128, 512], f32, name="psv", tag="aux")
                    for kt in range(KT):
                        nc.tensor.matmul(
                            ps[:, 0:CH],
                            lhsT=xt[:, kt, tt * 128 : (tt + 1) * 128],
                            rhs=wv_sb[:, kt, :],
                            start=(kt == 0),
                            stop=False,
                        )
                    nc.tensor.matmul(
                        ps[:, 0:CH],
                        lhsT=ones_sb[0:1, :],
                        rhs=wvb_sb[:],
                        start=False,
                        stop=True,
                    )
                    nc.vector.tensor_copy(v_sb[:, tt, :], ps[:, 0:CH])

                def transpose_pair(osb_t, qt, s):
                    """oT_sb[:, blk, qt*256+128s:+128] = osb_t[:, blk, :]^T."""
                    tr = aux.tile([128, 2, 128], bf16, name="tr", tag="aux")
                    for blk in range(2):
                        nc.tensor.transpose(
                            tr[:, blk, :], osb_t[:, blk, :], ident_sb[:]
                        )
                    for blk in range(2):
                        nc.vector.tensor_copy(
                            oT_sb[:, blk, qt * QB + 128 * s : qt * QB + 128 * (s + 1)],
                            tr[:, blk, :],
                        )

                def outproj(tt):
                    """out[tt*128:+128, :] = sum_ct oT_ct^T @ Wo_ct."""
                    stg = stgp.tile([128, D], bf16, name="stg", tag="stg")
                    for nt in range(2):
                        ps = aux.tile([128, 512], f32, name="ops", tag="aux")
                        for ct in range(2):
                            nc.tensor.matmul(
                                ps[:],
                                lhsT=oT_sb[:, ct, tt * 128 : (tt + 1) * 128],
                                rhs=wo_sb[:, ct, nt * 512 : (nt + 1) * 512],
                                start=(ct == 0),
                                stop=(ct == 1),
                            )
                        nc.vector.tensor_copy(stg[:, nt * 512 : (nt + 1) * 512], ps[:])
                    nc.sync.dma_start(out[tt * 128 : (tt + 1) * 128, :], stg[:])

                def emit(item):
                    kind = item[0]
                    if kind == "q":
                        qk_proj(qT_sb, wq_sb, 0, item[1], item[2])
                    elif kind == "k":
                        qk_proj(kT_sb, wk_sb, 2, item[1], item[2])
                    elif kind == "v":
                        v_proj(item[1])
                    elif kind == "tr":
                        transpose_pair(item[1], item[2], item[3])
                    elif kind == "o":
                        outproj(item[1])

                from collections import deque

                pending = deque()

                # front: projections for tokens 0-255 (x slice 0 only)
                for mt in range(2):
                    qk_proj(kT_sb, wk_sb, 2, mt, 0)
                for mt in range(2):
                    qk_proj(qT_sb, wq_sb, 0, mt, 0)
                v_proj(0)
                v_proj(1)

                def s_exp(qt, kt2):
                    """S^T + exp for (qt, kt2); returns the pt tile."""
                    st = stp.tile([128, NH, QB], f32, name="st", tag="st")
                    for h in range(NH):
                        a = h % 2
                        nc.tensor.matmul(
                            st[:, h, :],
                            lhsT=kT_sb[
                                64 * a : 64 * a + 64,
                                h // 2,
                                kt2 * 128 : (kt2 + 1) * 128,
                            ],
                            rhs=qT_sb[
                                64 * a : 64 * a + 64,
                                h // 2,
                                qt * QB : (qt + 1) * QB,
                            ],
                            start=(h % 2 == 0),  # bank opener per 2 heads
                            stop=True,
                            skip_group_check=True,
                        )
                    pt = ptp.tile([128, NH, QB], bf16, name="pt", tag="pt")
                    nc.scalar.activation(pt[:], st[:], Exp, scale=SCALE)
                    return pt

                pt_next = s_exp(0, 0)
                for qt in range(NQT):
                    o_ps = op_.tile([128, 2, NH, SUB], f32, name="o_ps", tag="o")
                    sm_ps = smp.tile([128, 512], f32, name="sm_ps", tag="sm")
                    for kt2 in range(TOKT):
                        first, last = kt2 == 0, kt2 == TOKT - 1
                        pt = pt_next
                        # software pipeline: S/exp of the NEXT tile go ahead of
                        # this tile's PV so the ACT stream never waits on PE
                        if not last:
                            pt_next = s_exp(qt, kt2 + 1)
                        elif qt + 1 < NQT:
                            pt_next = s_exp(qt + 1, 0)
                        # high-priority streamed projections, placed before PV
                        # (PV may lag ACT thanks to the deep pt pool)
                        if qt == 0:
                            # all of kT and v must materialize inside qt0
                            s_next = kt2 // 2 + 1
                            if kt2 % 2 == 0 and s_next < 8:
                                qk_proj(kT_sb, wk_sb, 2, 0, s_next)
                                qk_proj(kT_sb, wk_sb, 2, 1, s_next)
                            if kt2 + 2 < TOKT:
                                v_proj(kt2 + 2)
                            elif kt2 == TOKT - 2:
                                v_proj(TOKT - 1)
                        if kt2 == 12 and qt + 1 < NQT:
                            qk_proj(qT_sb, wq_sb, 0, 0, qt + 1)
                        if kt2 == 13 and qt + 1 < NQT:
                            qk_proj(qT_sb, wq_sb, 0, 1, qt + 1)
                        if kt2 >= 2 and pending:
                            emit(pending.popleft())
                        for s in range(2):
                            for h in range(NH):
                                nc.tensor.matmul(
                                    o_ps[:, s, h, :],
                                    lhsT=pt[:, h, 128 * s : 128 * (s + 1)],
                                    rhs=v_sb[:, kt2, 64 * h : 64 * h + 64],
                                    start=(first and s == 0 and h == 0),
                                    stop=last,
                                    skip_group_check=True,
                                )
                                nc.tensor.matmul(
                                    sm_ps[:, 4 * s + h : 4 * s + h + 1],
                                    lhsT=pt[:, h, 128 * s : 128 * (s + 1)],
                                    rhs=ones_sb[:, 0:1],
                                    start=(first and s == 0 and h == 0),
                                    stop=last,
                                    skip_group_check=True,
                                )
                    rcp = rcpp.tile([128, 8], f32, name="rcp", tag="rcp")
                    nc.vector.reciprocal(rcp[:], sm_ps[:, 0:8])
                    ot = []
                    for s in range(2):
                        osb_t = osb.tile([128, 2, 128], bf16, name="osb", tag="osb")
                        ot.append(osb_t)
                        for h in range(NH):
                            nc.vector.tensor_scalar_mul(
                                osb_t[:, h // 2, 64 * (h % 2) : 64 * (h % 2) + 64],
                                o_ps[:, s, h, :],
                                rcp[:, 4 * s + h : 4 * s + h + 1],
                            )
                    # post-qt work runs during the next qt's ACT windows
                    pending.append(("tr", ot[0], qt, 0))
                    pending.append(("tr", ot[1], qt, 1))
                    pending.append(("o", 2 * qt))
                    pending.append(("o", 2 * qt + 1))
                while pending:
                    emit(pending.popleft())
    nc.finalize()
    return nc


def make_in_maps(x, W_qkv, b_qkv, W_o):
    """Shard full inputs into per-core input maps (core c: batch c//4, group c%4)."""
    import ml_dtypes

    bf16 = ml_dtypes.bfloat16
    x = np.asarray(x, dtype=np.float32)
    W_qkv = np.asarray(W_qkv, dtype=np.float32)
    b_qkv = np.asarray(b_qkv, dtype=np.float32)
    W_o = np.asarray(W_o, dtype=np.float32)

    def pack_w(w):  # [1024, CH] -> [128, KT, CH] partition-major
        return np.ascontiguousarray(
            w.reshape(KT, 128, CH).transpose(1, 0, 2).astype(bf16)
        )

    in_maps = []
    for c in range(NCORES):
        b, g = divmod(c, GROUPS)
        cols = slice(CH * g, CH * (g + 1))
        bq = b_qkv[0 * D : 1 * D][cols]
        bk = b_qkv[1 * D : 2 * D][cols]
        bqk = np.stack(
            [bq[0:128], bq[128:256], bk[0:128], bk[128:256]], axis=1
        ).astype(np.float32)
        xh = (
            x[b].T.reshape(KT, 128, N).transpose(1, 0, 2).astype(bf16)
        )  # [128, KT, N]
        m = {
            "xh": np.ascontiguousarray(xh),
            "wq": pack_w(W_qkv[:, 0 * D : 1 * D][:, cols]),
            "wk": pack_w(W_qkv[:, 1 * D : 2 * D][:, cols]),
            "wv": pack_w(W_qkv[:, 2 * D : 3 * D][:, cols]),
            "wvb": np.ascontiguousarray(
                b_qkv[2 * D : 3 * D][cols][None, :].astype(bf16)
            ),
            "wo": np.ascontiguousarray(
                W_o[cols, :].reshape(2, 128, D).transpose(1, 0, 2).astype(bf16)
            ),
            "bqk": np.ascontiguousarray(bqk),
            "ones": np.ones((128, 128), dtype=bf16),
            "ident": np.eye(128, dtype=bf16),
        }
        in_maps.append(m)
    return in_maps


_NC = None


def get_nc():
    global _NC
    if _NC is None:
        _NC = build_nc()
    return _NC


def kernel(x, W_qkv, b_qkv, W_o, b_o):
    from concourse import bass_utils

    b_o = np.asarray(b_o, dtype=np.float32)
    in_maps = make_in_maps(x, W_qkv, b_qkv, W_o)
    res = bass_utils.run_bass_kernel_spmd(get_nc(), in_maps, core_ids=list(range(NCORES)))
    out = np.empty((B, N, D), dtype=np.float32)
    for b in range(B):
        acc = res.results[4 * b]["out"].astype(np.float32)
        for g in range(1, GROUPS):
            acc += res.results[4 * b + g]["out"].astype(np.float32)
        out[b] = acc + b_o
    return out


# revision 10
# speedup vs baseline: 1.5784x; 1.0146x over previous
"""Multi-head attention (B=2, N=2048, D=1024, H=16) on 8 Trainium2 cores.

Sharding: data-parallel over batch (2) x tensor-parallel over head groups (4).
Core c handles batch c//4, heads 4*(c%4) .. 4*(c%4)+3.

Per-core kernel, all matmul operands bf16 (rate 1.0 cycles/row at any width):
  front:   kT/qT = (W^T @ x^T) + bias   (channels on partitions)
           v     = ([x;1] @ [Wv;bv])    (tokens on partitions)
  per query-tile of 256 (ACT-exp paces at ~1.04us/key-ptile):
    per key-ptile kt2 (128 keys):
      S^T[:,h,:] = kT_h^T qT_h          (keys on partitions, 4 matmuls)
      P^T = exp(SCALE * S^T)            (one ACT op over all 4 heads)
      O[q,s,h]  += P^T_slice^T @ v_h    (64-col streams, queries on partitions)
      sums[q,h] += P^T_slice^T @ 1      (1-col streams, ~free)
    O /= sums (DVE per-partition scalars) -> transpose back (PE, identity)
    out[tokens] = sum_ct oT_ct^T @ Wo_ct  (chains of 2, K=128 each)
Host: out[b] = sum of the 4 group partials + b_o.
"""

import sys

sys.path.insert(0, "/opt/trn_rl_repo")

import numpy as np

B, N, D, H = 2, 2048, 1024, 16
SUB = D // H  # 64
GROUPS = 4  # tensor-parallel head groups
NH = H // GROUPS  # 4 local heads per core
CH = NH * SUB  # 256 local channels
NCORES = 8
QB = 256  # query tile
NQT = N // QB  # 8
KT = D // 128  # contraction ptiles
TOKT = N // 128  # token/key ptiles
SCALE = SUB ** -0.5


def build_nc(name="mha"):
    import concourse.mybir as mybir
    from concourse import bacc
    from concourse.tile import TileContext

    f32 = mybir.dt.float32
    bf16 = mybir.dt.bfloat16
    Exp = mybir.ActivationFunctionType.Exp

    nc = bacc.Bacc(None, name=name)
    # host-packed, partition-major layouts (see make_in_maps)
    xh = nc.dram_tensor("xh", [128, KT, N], bf16, kind="ExternalInput")
    wq = nc.dram_tensor("wq", [128, KT, CH], bf16, kind="ExternalInput")
    wk = nc.dram_tensor("wk", [128, KT, CH], bf16, kind="ExternalInput")
    wv = nc.dram_tensor("wv", [128, KT, CH], bf16, kind="ExternalInput")
    wvb = nc.dram_tensor("wvb", [1, CH], bf16, kind="ExternalInput")
    wo = nc.dram_tensor("wo", [128, 2, D], bf16, kind="ExternalInput")
    bqk = nc.dram_tensor("bqk", [128, 4], f32, kind="ExternalInput")
    ones_d = nc.dram_tensor("ones", [128, 128], bf16, kind="ExternalInput")
    ident_d = nc.dram_tensor("ident", [128, 128], bf16, kind="ExternalInput")
    out = nc.dram_tensor("out", [N, D], bf16, kind="ExternalOutput")

    with TileContext(nc) as tc:
        with tc.tile_pool(name="persist", bufs=1) as pp:
            xt = pp.tile([128, KT, N], bf16)
            wq_sb = pp.tile([128, KT, CH], bf16)
            wk_sb = pp.tile([128, KT, CH], bf16)
            wv_sb = pp.tile([128, KT, CH], bf16)
            wvb_sb = pp.tile([1, CH], bf16)
            wo_sb = pp.tile([128, 2, D], bf16)
            qT_sb = pp.tile([128, 2, N], bf16)
            kT_sb = pp.tile([128, 2, N], bf16)
            v_sb = pp.tile([128, TOKT, CH], bf16)
            oT_sb = pp.tile([128, 2, N], bf16)
            bqk_sb = pp.tile([128, 4], f32)
            ones_sb = pp.tile([128, 128], bf16)
            ident_sb = pp.tile([128, 128], bf16)

            # DMA issue order: earliest-needed first. x comes in 4 token
            # quarters so the first projections can start at ~4.5us.
            nc.sync.dma_start(wk_sb[:], wk[:])
            nc.sync.dma_start(xt[:, :, 0:256], xh[:, :, 0:256])
            nc.sync.dma_start(wq_sb[:], wq[:])
            nc.sync.dma_start(xt[:, :, 256:512], xh[:, :, 256:512])
            nc.sync.dma_start(wv_sb[:], wv[:])
            nc.sync.dma_start(wvb_sb[:], wvb[:])
            nc.sync.dma_start(bqk_sb[:], bqk[:])
            nc.sync.dma_start(ones_sb[:], ones_d[:])
            nc.sync.dma_start(xt[:, :, 512:768], xh[:, :, 512:768])
            nc.sync.dma_start(ident_sb[:], ident_d[:])
            nc.sync.dma_start(xt[:, :, 768:1024], xh[:, :, 768:1024])
            nc.sync.dma_start(wo_sb[:], wo[:])
            for sl in range(4, 8):
                nc.sync.dma_start(
                    xt[:, :, sl * 256 : (sl + 1) * 256],
                    xh[:, :, sl * 256 : (sl + 1) * 256],
                )

            with tc.tile_pool(name="stp", bufs=2, space="PSUM") as stp, \
                 tc.tile_pool(name="op_", bufs=1, space="PSUM") as op_, \
                 tc.tile_pool(name="smp", bufs=1, space="PSUM") as smp, \
                 tc.tile_pool(name="aux", bufs=2, space="PSUM") as aux, \
                 tc.tile_pool(name="ptp", bufs=8) as ptp, \
                 tc.tile_pool(name="osb", bufs=3) as osb, \
                 tc.tile_pool(name="rcpp", bufs=2) as rcpp, \
                 tc.tile_pool(name="stg", bufs=2) as stgp:

                def qk_proj(dst, wt, bcol, mt, s):
                    """dst[:, mt, 256s:+256] = (W^T x^T)[128ch, 256tok] + bias."""
                    ps = aux.tile([128, 512], f32, name="ps", tag="aux")
                    for kt in range(KT):
                        nc.tensor.matmul(
                            ps[:, 0:QB],
                            lhsT=wt[:, kt, mt * 128 : (mt + 1) * 128],
                            rhs=xt[:, kt, s * QB : (s + 1) * QB],
                            start=(kt == 0),
                            stop=(kt == KT - 1),
                        )
                    nc.vector.tensor_scalar_add(
                        dst[:, mt, s * QB : (s + 1) * QB],
                        ps[:, 0:QB],
                        bqk_sb[:, bcol + mt : bcol + mt + 1],
                    )

                def v_proj(tt):
                    """v_sb[:, tt, :] = ([x;1] @ [Wv;bv])[128tok, 256ch]."""
                    ps = aux.tile([128, 512], f32, name="psv", tag="aux")
                    for kt in range(KT):
                        nc.tensor.matmul(
                            ps[:, 0:CH],
                            lhsT=xt[:, kt, tt * 128 : (tt + 1) * 128],
                            rhs=wv_sb[:, kt, :],
                            start=(kt == 0),
                            stop=False,
                        )
                    nc.tensor.matmul(
                        ps[:, 0:CH],
                        lhsT=ones_sb[0:1, :],
                        rhs=wvb_sb[:],
                        start=False,
                        stop=True,
                    )
                    nc.vector.tensor_copy(v_sb[:, tt, :], ps[:, 0:CH])

                def transpose_pair(osb_t, qt, s):
                    """oT_sb[:, blk, qt*256+128s:+128] = osb_t[:, blk, :]^T."""
                    tr = aux.tile([128, 2, 128], bf16, name="tr", tag="aux")
                    for blk in range(2):
                        nc.tensor.transpose(
                            tr[:, blk, :], osb_t[:, blk, :], ident_sb[:]
                        )
                    for blk in range(2):
                        nc.vector.tensor_copy(
                            oT_sb[:, blk, qt * QB + 128 * s : qt * QB + 128 * (s + 1)],
                            tr[:, blk, :],
                        )

                def outproj(tt):
                    """out[tt*128:+128, :] = sum_ct oT_ct^T @ Wo_ct."""
                    stg = stgp.tile([128, D], bf16, name="stg", tag="stg")
                    for nt in range(2):
                        ps = aux.tile([128, 512], f32, name="ops", tag="aux")
                        for ct in range(2):
                            nc.tensor.matmul(
                                ps[:],
                                lhsT=oT_sb[:, ct, tt * 128 : (tt + 1) * 128],
                                rhs=wo_sb[:, ct, nt * 512 : (nt + 1) * 512],
                                start=(ct == 0),
                                stop=(ct == 1),
                            )
                        nc.vector.tensor_copy(stg[:, nt * 512 : (nt + 1) * 512], ps[:])
                    nc.sync.dma_start(out[tt * 128 : (tt + 1) * 128, :], stg[:])

                def emit(item):
                    kind = item[0]
                    if kind == "q":
                        qk_proj(qT_sb, wq_sb, 0, item[1], item[2])
                    elif kind == "k":
                        qk_proj(kT_sb, wk_sb, 2, item[1], item[2])
                    elif kind == "v":
                        v_proj(item[1])
                    elif kind == "tr":
                        transpose_pair(item[1], item[2], item[3])
                    elif kind == "o":
                        outproj(item[1])

                from collections import deque

                pending = deque()

                # PE p-state warmup: keep the PE continuously busy from t~0 so
                # it reaches full clock (3us ramp) before the real work lands.
                warm = pp.tile([128, 256], bf16)
                nc.vector.memset(warm[:], 0.0)
                wps = aux.tile([128, 512], f32, name="wps", tag="aux")
                for i in range(14):
                    nc.tensor.matmul(
                        wps[:, 0:256],
                        lhsT=warm[:, 0:128],
                        rhs=warm[:, :],
                        start=True,
                        stop=True,
                        skip_group_check=True,
                    )

                # front: projections for tokens 0-255 (x slice 0 only)
                for mt in range(2):
                    qk_proj(kT_sb, wk_sb, 2, mt, 0)
                for mt in range(2):
                    qk_proj(qT_sb, wq_sb, 0, mt, 0)
                v_proj(0)
                v_proj(1)

                def s_exp(qt, kt2):
                    """S^T + exp for (qt, kt2); returns the pt tile."""
                    st = stp.tile([128, NH, QB], f32, name="st", tag="st")
                    for h in range(NH):
                        a = h % 2
                        nc.tensor.matmul(
                            st[:, h, :],
                            lhsT=kT_sb[
                                64 * a : 64 * a + 64,
                                h // 2,
                                kt2 * 128 : (kt2 + 1) * 128,
                            ],
                            rhs=qT_sb[
                                64 * a : 64 * a + 64,
                                h // 2,
                                qt * QB : (qt + 1) * QB,
                            ],
                            start=(h % 2 == 0),  # bank opener per 2 heads
                            stop=True,
                            skip_group_check=True,
                        )
                    pt = ptp.tile([128, NH, QB], bf16, name="pt", tag="pt")
                    nc.scalar.activation(pt[:], st[:], Exp, scale=SCALE)
                    return pt

                def pv(pt, kt2, o_ps, sm_ps):
                    first, last = kt2 == 0, kt2 == TOKT - 1
                    for s in range(2):
                        for h in range(NH):
                            nc.tensor.matmul(
                                o_ps[:, s, h, :],
                                lhsT=pt[:, h, 128 * s : 128 * (s + 1)],
                                rhs=v_sb[:, kt2, 64 * h : 64 * h + 64],
                                start=(first and s == 0 and h == 0),
                                stop=last,
                                skip_group_check=True,
                            )
                            nc.tensor.matmul(
                                sm_ps[:, 4 * s + h : 4 * s + h + 1],
                                lhsT=pt[:, h, 128 * s : 128 * (s + 1)],
                                rhs=ones_sb[:, 0:1],
                                start=(first and s == 0 and h == 0),
                                stop=last,
                                skip_group_check=True,
                            )

                pt_next = s_exp(0, 0)
                pv_q = deque()
                for qt in range(NQT):
                    o_ps = op_.tile([128, 2, NH, SUB], f32, name="o_ps", tag="o")
                    sm_ps = smp.tile([128, 512], f32, name="sm_ps", tag="sm")
                    for kt2 in range(TOKT):
                        last = kt2 == TOKT - 1
                        pv_q.append((pt_next, kt2, o_ps, sm_ps))
                        # software pipeline: S/exp of the NEXT tile go ahead of
                        # this tile's PV so the ACT stream never waits on PE
                        if not last:
                            pt_next = s_exp(qt, kt2 + 1)
                        elif qt + 1 < NQT:
                            pt_next = s_exp(qt + 1, 0)
                        # high-priority streamed projections, placed before PV
                        # (PV lags ACT by one tile thanks to the deep pt pool)
                        if qt == 0:
                            # all of kT and v must materialize inside qt0
                            s_next = kt2 // 2 + 1
                            if kt2 % 2 == 0 and s_next < 8:
                                qk_proj(kT_sb, wk_sb, 2, 0, s_next)
                                qk_proj(kT_sb, wk_sb, 2, 1, s_next)
                            if kt2 + 2 < TOKT:
                                v_proj(kt2 + 2)
                            elif kt2 == TOKT - 2:
                                v_proj(TOKT - 1)
                        if kt2 == 12 and qt + 1 < NQT:
                            qk_proj(qT_sb, wq_sb, 0, 0, qt + 1)
                        if kt2 == 13 and qt + 1 < NQT:
                            qk_proj(qT_sb, wq_sb, 0, 1, qt + 1)
                        if kt2 >= 2 and pending:
                            emit(pending.popleft())
                        if len(pv_q) > 1:
                            pv(*pv_q.popleft())
                    while pv_q:
                        pv(*pv_q.popleft())
                    rcp = rcpp.tile([128, 8], f32, name="rcp", tag="rcp")
                    nc.vector.reciprocal(rcp[:], sm_ps[:, 0:8])
                    ot = []
                    for s in range(2):
                        osb_t = osb.tile([128, 2, 128], bf16, name="osb", tag="osb")
                        ot.append(osb_t)
                        for h in range(NH):
                            nc.vector.tensor_scalar_mul(
                                osb_t[:, h // 2, 64 * (h % 2) : 64 * (h % 2) + 64],
                                o_ps[:, s, h, :],
                                rcp[:, 4 * s + h : 4 * s + h + 1],
                            )
                    # post-qt work runs during the next qt's ACT windows
                    pending.append(("tr", ot[0], qt, 0))
                    pending.append(("tr", ot[1], qt, 1))
                    pending.append(("o", 2 * qt))
                    pending.append(("o", 2 * qt + 1))
                while pending:
                    emit(pending.popleft())
    nc.finalize()
    return nc


def make_in_maps(x, W_qkv, b_qkv, W_o):
    """Shard full inputs into per-core input maps (core c: batch c//4, group c%4)."""
    import ml_dtypes

    bf16 = ml_dtypes.bfloat16
    x = np.asarray(x, dtype=np.float32)
    W_qkv = np.asarray(W_qkv, dtype=np.float32)
    b_qkv = np.asarray(b_qkv, dtype=np.float32)
    W_o = np.asarray(W_o, dtype=np.float32)

    def pack_w(w):  # [1024, CH] -> [128, KT, CH] partition-major
        return np.ascontiguousarray(
            w.reshape(KT, 128, CH).transpose(1, 0, 2).astype(bf16)
        )

    in_maps = []
    for c in range(NCORES):
        b, g = divmod(c, GROUPS)
        cols = slice(CH * g, CH * (g + 1))
        bq = b_qkv[0 * D : 1 * D][cols]
        bk = b_qkv[1 * D : 2 * D][cols]
        bqk = np.stack(
            [bq[0:128], bq[128:256], bk[0:128], bk[128:256]], axis=1
        ).astype(np.float32)
        xh = (
            x[b].T.reshape(KT, 128, N).transpose(1, 0, 2).astype(bf16)
        )  # [128, KT, N]
        m = {
            "xh": np.ascontiguousarray(xh),
            "wq": pack_w(W_qkv[:, 0 * D : 1 * D][:, cols]),
            "wk": pack_w(W_qkv[:, 1 * D : 2 * D][:, cols]),
            "wv": pack_w(W_qkv[:, 2 * D : 3 * D][:, cols]),
            "wvb": np.ascontiguousarray(
                b_qkv[2 * D : 3 * D][cols][None, :].astype(bf16)
            ),
            "wo": np.ascontiguousarray(
                W_o[cols, :].reshape(2, 128, D).transpose(1, 0, 2).astype(bf16)
            ),
            "bqk": np.ascontiguousarray(bqk),
            "ones": np.ones((128, 128), dtype=bf16),
            "ident": np.eye(128, dtype=bf16),
        }
        in_maps.append(m)
    return in_maps


_NC = None


def get_nc():
    global _NC
    if _NC is None:
        _NC = build_nc()
    return _NC


def kernel(x, W_qkv, b_qkv, W_o, b_o):
    from concourse import bass_utils

    b_o = np.asarray(b_o, dtype=np.float32)
    in_maps = make_in_maps(x, W_qkv, b_qkv, W_o)
    res = bass_utils.run_bass_kernel_spmd(get_nc(), in_maps, core_ids=list(range(NCORES)))
    out = np.empty((B, N, D), dtype=np.float32)
    for b in range(B):
        acc = res.results[4 * b]["out"].astype(np.float32)
        for g in range(1, GROUPS):
            acc += res.results[4 * b + g]["out"].astype(np.float32)
        out[b] = acc + b_o
    return out


# revision 14
# speedup vs baseline: 1.6274x; 1.0310x over previous
"""Multi-head attention (B=2, N=2048, D=1024, H=16) on 8 Trainium2 cores.

Sharding: data-parallel over batch (2) x tensor-parallel over head groups (4).
Core c handles batch c//4, heads 4*(c%4) .. 4*(c%4)+3.

Per-core kernel, all matmul operands bf16 (rate 1.0 cycles/row at any width):
  front:   kT/qT = (W^T @ x^T) + bias   (channels on partitions)
           v     = ([x;1] @ [Wv;bv])    (tokens on partitions)
  per query-tile of 256 (ACT-exp paces at ~1.04us/key-ptile):
    per key-ptile kt2 (128 keys):
      S^T[:,h,:] = kT_h^T qT_h          (keys on partitions, 4 matmuls)
      P^T = exp(SCALE * S^T)            (one ACT op over all 4 heads)
      O[q,s,h]  += P^T_slice^T @ v_h    (64-col streams, queries on partitions)
      sums[q,h] += P^T_slice^T @ 1      (1-col streams, ~free)
    O /= sums (DVE per-partition scalars) -> transpose back (PE, identity)
    out[tokens] = sum_ct oT_ct^T @ Wo_ct  (chains of 2, K=128 each)
Host: out[b] = sum of the 4 group partials + b_o.
"""

import sys

sys.path.insert(0, "/opt/trn_rl_repo")

import numpy as np

B, N, D, H = 2, 2048, 1024, 16
SUB = D // H  # 64
GROUPS = 4  # tensor-parallel head groups
NH = H // GROUPS  # 4 local heads per core
CH = NH * SUB  # 256 local channels
NCORES = 8
QB = 256  # query tile
NQT = N // QB  # 8
KT = D // 128  # contraction ptiles
TOKT = N // 128  # token/key ptiles
SCALE = SUB ** -0.5


def build_nc(name="mha"):
    import concourse.mybir as mybir
    from concourse import bacc
    from concourse.tile import TileContext

    f32 = mybir.dt.float32
    bf16 = mybir.dt.bfloat16
    Exp = mybir.ActivationFunctionType.Exp

    nc = bacc.Bacc(None, name=name)
    # host-packed, partition-major layouts (see make_in_maps)
    xh = nc.dram_tensor("xh", [128, KT, N], bf16, kind="ExternalInput")
    wq = nc.dram_tensor("wq", [128, KT, CH], bf16, kind="ExternalInput")
    wk = nc.dram_tensor("wk", [128, KT, CH], bf16, kind="ExternalInput")
    wv = nc.dram_tensor("wv", [128, KT, CH], bf16, kind="ExternalInput")
    wvb = nc.dram_tensor("wvb", [1, CH], bf16, kind="ExternalInput")
    wo = nc.dram_tensor("wo", [128, 2, D], bf16, kind="ExternalInput")
    bqk = nc.dram_tensor("bqk", [128, 4], f32, kind="ExternalInput")
    ones_d = nc.dram_tensor("ones", [128, 128], bf16, kind="ExternalInput")
    ident_d = nc.dram_tensor("ident", [128, 128], bf16, kind="ExternalInput")
    out = nc.dram_tensor("out", [N, D], bf16, kind="ExternalOutput")

    with TileContext(nc) as tc:
        with tc.tile_pool(name="persist", bufs=1) as pp:
            xt = pp.tile([128, KT, N], bf16)
            wq_sb = pp.tile([128, KT, CH], bf16)
            wk_sb = pp.tile([128, KT, CH], bf16)
            wv_sb = pp.tile([128, KT, CH], bf16)
            wvb_sb = pp.tile([1, CH], bf16)
            wo_sb = pp.tile([128, 2, D], bf16)
            qT_sb = pp.tile([128, 2, N], bf16)
            kT_sb = pp.tile([128, 2, N], bf16)
            v_sb = pp.tile([128, TOKT, CH], bf16)
            oT_sb = pp.tile([128, 2, N], bf16)
            bqk_sb = pp.tile([128, 4], f32)
            ones_sb = pp.tile([128, 128], bf16)
            ident_sb = pp.tile([128, 128], bf16)

            # DMA issue order: earliest-needed first. x comes in 4 token
            # quarters so the first projections can start at ~4.5us.
            nc.sync.dma_start(wk_sb[:], wk[:])
            nc.sync.dma_start(xt[:, :, 0:256], xh[:, :, 0:256])
            nc.sync.dma_start(bqk_sb[:], bqk[:])
            nc.sync.dma_start(wq_sb[:], wq[:])
            nc.sync.dma_start(wv_sb[:], wv[:])
            nc.sync.dma_start(wvb_sb[:], wvb[:])
            nc.sync.dma_start(ones_sb[:], ones_d[:])
            nc.sync.dma_start(xt[:, :, 256:512], xh[:, :, 256:512])
            nc.sync.dma_start(ident_sb[:], ident_d[:])
            nc.sync.dma_start(xt[:, :, 512:768], xh[:, :, 512:768])
            nc.sync.dma_start(xt[:, :, 768:1024], xh[:, :, 768:1024])
            nc.sync.dma_start(wo_sb[:], wo[:])
            for sl in range(4, 8):
                nc.sync.dma_start(
                    xt[:, :, sl * 256 : (sl + 1) * 256],
                    xh[:, :, sl * 256 : (sl + 1) * 256],
                )

            with tc.tile_pool(name="stp", bufs=2, space="PSUM") as stp, \
                 tc.tile_pool(name="op_", bufs=1, space="PSUM") as op_, \
                 tc.tile_pool(name="smp", bufs=1, space="PSUM") as smp, \
                 tc.tile_pool(name="aux", bufs=2, space="PSUM") as aux, \
                 tc.tile_pool(name="ptp", bufs=8) as ptp, \
                 tc.tile_pool(name="osb", bufs=3) as osb, \
                 tc.tile_pool(name="rcpp", bufs=2) as rcpp, \
                 tc.tile_pool(name="stg", bufs=2) as stgp:

                def qk_proj(dst, wt, bcol, mt, s):
                    """dst[:, mt, 256s:+256] = (W^T x^T)[128ch, 256tok] + bias."""
                    ps = aux.tile([128, 512], f32, name="ps", tag="aux")
                    for kt in range(KT):
                        nc.tensor.matmul(
                            ps[:, 0:QB],
                            lhsT=wt[:, kt, mt * 128 : (mt + 1) * 128],
                            rhs=xt[:, kt, s * QB : (s + 1) * QB],
                            start=(kt == 0),
                            stop=(kt == KT - 1),
                        )
                    nc.vector.tensor_scalar_add(
                        dst[:, mt, s * QB : (s + 1) * QB],
                        ps[:, 0:QB],
                        bqk_sb[:, bcol + mt : bcol + mt + 1],
                    )

                def v_proj(tt):
                    """v_sb[:, tt, :] = ([x;1] @ [Wv;bv])[128tok, 256ch]."""
                    ps = aux.tile([128, 512], f32, name="psv", tag="aux")
                    for kt in range(KT):
                        nc.tensor.matmul(
                            ps[:, 0:CH],
                            lhsT=xt[:, kt, tt * 128 : (tt + 1) * 128],
                            rhs=wv_sb[:, kt, :],
                            start=(kt == 0),
                            stop=False,
                        )
                    nc.tensor.matmul(
                        ps[:, 0:CH],
                        lhsT=ones_sb[0:1, :],
                        rhs=wvb_sb[:],
                        start=False,
                        stop=True,
                    )
                    nc.vector.tensor_copy(v_sb[:, tt, :], ps[:, 0:CH])

                def transpose_pair(osb_t, qt, s):
                    """oT_sb[:, blk, qt*256+128s:+128] = osb_t[:, blk, :]^T."""
                    tr = aux.tile([128, 2, 128], bf16, name="tr", tag="aux")
                    for blk in range(2):
                        nc.tensor.transpose(
                            tr[:, blk, :], osb_t[:, blk, :], ident_sb[:]
                        )
                    for blk in range(2):
                        nc.vector.tensor_copy(
                            oT_sb[:, blk, qt * QB + 128 * s : qt * QB + 128 * (s + 1)],
                            tr[:, blk, :],
                        )

                def outproj(tt):
                    """out[tt*128:+128, :] = sum_ct oT_ct^T @ Wo_ct."""
                    stg = stgp.tile([128, D], bf16, name="stg", tag="stg")
                    for nt in range(2):
                        ps = aux.tile([128, 512], f32, name="ops", tag="aux")
                        for ct in range(2):
                            nc.tensor.matmul(
                                ps[:],
                                lhsT=oT_sb[:, ct, tt * 128 : (tt + 1) * 128],
                                rhs=wo_sb[:, ct, nt * 512 : (nt + 1) * 512],
                                start=(ct == 0),
                                stop=(ct == 1),
                            )
                        nc.vector.tensor_copy(stg[:, nt * 512 : (nt + 1) * 512], ps[:])
                    nc.sync.dma_start(out[tt * 128 : (tt + 1) * 128, :], stg[:])

                def emit(item):
                    kind = item[0]
                    if kind == "q":
                        qk_proj(qT_sb, wq_sb, 0, item[1], item[2])
                    elif kind == "k":
                        qk_proj(kT_sb, wk_sb, 2, item[1], item[2])
                    elif kind == "v":
                        v_proj(item[1])
                    elif kind == "tr":
                        transpose_pair(item[1], item[2], item[3])
                    elif kind == "o":
                        outproj(item[1])

                from collections import deque

                pending = deque()

                # PE p-state warmup: keep the PE continuously busy from t~0 so
                # it reaches full clock (3us ramp) before the real work lands.
                warm = pp.tile([128, 256], bf16)
                nc.vector.memset(warm[:], 0.0)
                wps = aux.tile([128, 512], f32, name="wps", tag="aux")
                for i in range(14):
                    nc.tensor.matmul(
                        wps[:, 0:256],
                        lhsT=warm[:, 0:128],
                        rhs=warm[:, :],
                        start=True,
                        stop=True,
                        skip_group_check=True,
                    )

                # front: projections for tokens 0-255 (x slice 0 only);
                # v goes after the first S/exp (emitted below) so the late
                # wv DMA never blocks the ACT stream start
                for mt in range(2):
                    qk_proj(kT_sb, wk_sb, 2, mt, 0)
                for mt in range(2):
                    qk_proj(qT_sb, wq_sb, 0, mt, 0)

                def s_exp(qt, kt2):
                    """S^T + exp for (qt, kt2); returns the pt tile."""
                    st = stp.tile([128, NH, QB], f32, name="st", tag="st")
                    for h in range(NH):
                        a = h % 2
                        nc.tensor.matmul(
                            st[:, h, :],
                            lhsT=kT_sb[
                                64 * a : 64 * a + 64,
                                h // 2,
                                kt2 * 128 : (kt2 + 1) * 128,
                            ],
                            rhs=qT_sb[
                                64 * a : 64 * a + 64,
                                h // 2,
                                qt * QB : (qt + 1) * QB,
                            ],
                            start=(h % 2 == 0),  # bank opener per 2 heads
                            stop=True,
                            skip_group_check=True,
                        )
                    pt = ptp.tile([128, NH, QB], bf16, name="pt", tag="pt")
                    nc.scalar.activation(pt[:], st[:], Exp, scale=SCALE)
                    return pt

                def pv(pt, kt2, o_ps, sm_ps):
                    first, last = kt2 == 0, kt2 == TOKT - 1
                    for s in range(2):
                        for h in range(NH):
                            nc.tensor.matmul(
                                o_ps[:, s, h, :],
                                lhsT=pt[:, h, 128 * s : 128 * (s + 1)],
                                rhs=v_sb[:, kt2, 64 * h : 64 * h + 64],
                                start=(first and s == 0 and h == 0),
                                stop=last,
                                skip_group_check=True,
                            )
                            nc.tensor.matmul(
                                sm_ps[:, 4 * s + h : 4 * s + h + 1],
                                lhsT=pt[:, h, 128 * s : 128 * (s + 1)],
                                rhs=ones_sb[:, 0:1],
                                start=(first and s == 0 and h == 0),
                                stop=last,
                                skip_group_check=True,
                            )

                pt_next = s_exp(0, 0)
                v_proj(0)
                v_proj(1)
                pv_q = deque()
                for qt in range(NQT):
                    o_ps = op_.tile([128, 2, NH, SUB], f32, name="o_ps", tag="o")
                    sm_ps = smp.tile([128, 512], f32, name="sm_ps", tag="sm")
                    for kt2 in range(TOKT):
                        last = kt2 == TOKT - 1
                        pv_q.append((pt_next, kt2, o_ps, sm_ps))
                        # software pipeline: S/exp of the NEXT tile go ahead of
                        # this tile's PV so the ACT stream never waits on PE
                        if not last:
                            pt_next = s_exp(qt, kt2 + 1)
                        elif qt + 1 < NQT:
                            pt_next = s_exp(qt + 1, 0)
                        # high-priority streamed projections, placed before PV
                        # (PV lags ACT by one tile thanks to the deep pt pool)
                        if qt == 0:
                            # all of kT and v must materialize inside qt0
                            s_next = kt2 // 2 + 1
                            if kt2 % 2 == 0 and s_next < 8:
                                qk_proj(kT_sb, wk_sb, 2, 0, s_next)
                                qk_proj(kT_sb, wk_sb, 2, 1, s_next)
                            if kt2 + 2 < TOKT:
                                v_proj(kt2 + 2)
                            elif kt2 == TOKT - 2:
                                v_proj(TOKT - 1)
                        if kt2 == 12 and qt + 1 < NQT:
                            qk_proj(qT_sb, wq_sb, 0, 0, qt + 1)
                        if kt2 == 13 and qt + 1 < NQT:
                            qk_proj(qT_sb, wq_sb, 0, 1, qt + 1)
                        if kt2 >= 2 and pending:
                            emit(pending.popleft())
                        # PV lags ACT by 2 tiles early in the qt (hides the
                        # O-bank reopen waiting on the previous normalize)
                        if len(pv_q) > (2 if kt2 < 8 else 1):
                            pv(*pv_q.popleft())
                    while pv_q:
                        pv(*pv_q.popleft())
                    rcp = rcpp.tile([128, 8], f32, name="rcp", tag="rcp")
                    for s in range(2):
                        nc.vector.reciprocal(
                            rcp[:, 4 * s : 4 * s + 4], sm_ps[:, 4 * s : 4 * s + 4]
                        )
                        osb_t = osb.tile([128, 2, 128], bf16, name="osb", tag="osb")
                        for h in range(NH):
                            nc.vector.tensor_scalar_mul(
                                osb_t[:, h // 2, 64 * (h % 2) : 64 * (h % 2) + 64],
                                o_ps[:, s, h, :],
                                rcp[:, 4 * s + h : 4 * s + h + 1],
                            )
                        # post-qt work runs during the next qt's ACT windows
                        pending.append(("tr", osb_t, qt, s))
                        pending.append(("o", 2 * qt + s))
                while pending:
                    emit(pending.popleft())
    nc.finalize()
    return nc


def make_in_maps(x, W_qkv, b_qkv, W_o):
    """Shard full inputs into per-core input maps (core c: batch c//4, group c%4)."""
    import ml_dtypes

    bf16 = ml_dtypes.bfloat16
    x = np.asarray(x, dtype=np.float32)
    W_qkv = np.asarray(W_qkv, dtype=np.float32)
    b_qkv = np.asarray(b_qkv, dtype=np.float32)
    W_o = np.asarray(W_o, dtype=np.float32)

    def pack_w(w):  # [1024, CH] -> [128, KT, CH] partition-major
        return np.ascontiguousarray(
            w.reshape(KT, 128, CH).transpose(1, 0, 2).astype(bf16)
        )

    in_maps = []
    for c in range(NCORES):
        b, g = divmod(c, GROUPS)
        cols = slice(CH * g, CH * (g + 1))
        bq = b_qkv[0 * D : 1 * D][cols]
        bk = b_qkv[1 * D : 2 * D][cols]
        bqk = np.stack(
            [bq[0:128], bq[128:256], bk[0:128], bk[128:256]], axis=1
        ).astype(np.float32)
        xh = (
            x[b].T.reshape(KT, 128, N).transpose(1, 0, 2).astype(bf16)
        )  # [128, KT, N]
        m = {
            "xh": np.ascontiguousarray(xh),
            "wq": pack_w(W_qkv[:, 0 * D : 1 * D][:, cols]),
            "wk": pack_w(W_qkv[:, 1 * D : 2 * D][:, cols]),
            "wv": pack_w(W_qkv[:, 2 * D : 3 * D][:, cols]),
            "wvb": np.ascontiguousarray(
                b_qkv[2 * D : 3 * D][cols][None, :].astype(bf16)
            ),
            "wo": np.ascontiguousarray(
                W_o[cols, :].reshape(2, 128, D).transpose(1, 0, 2).astype(bf16)
            ),
            "bqk": np.ascontiguousarray(bqk),
            "ones": np.ones((128, 128), dtype=bf16),
            "ident": np.eye(128, dtype=bf16),
        }
        in_maps.append(m)
    return in_maps


_NC = None


def get_nc():
    global _NC
    if _NC is None:
        _NC = build_nc()
    return _NC


def kernel(x, W_qkv, b_qkv, W_o, b_o):
    from concourse import bass_utils

    b_o = np.asarray(b_o, dtype=np.float32)
    in_maps = make_in_maps(x, W_qkv, b_qkv, W_o)
    res = bass_utils.run_bass_kernel_spmd(get_nc(), in_maps, core_ids=list(range(NCORES)))
    out = np.empty((B, N, D), dtype=np.float32)
    for b in range(B):
        acc = res.results[4 * b]["out"].astype(np.float32)
        for g in range(1, GROUPS):
            acc += res.results[4 * b + g]["out"].astype(np.float32)
        out[b] = acc + b_o
    return out


# revision 16
# speedup vs baseline: 1.6611x; 1.0207x over previous
"""Multi-head attention (B=2, N=2048, D=1024, H=16) on 8 Trainium2 cores.

Sharding: data-parallel over batch (2) x tensor-parallel over head groups (4).
Core c handles batch c//4, heads 4*(c%4) .. 4*(c%4)+3.

Per-core kernel, all matmul operands bf16 (rate 1.0 cycles/row at any width):
  front:   kT/qT = (W^T @ x^T) + bias   (channels on partitions)
           v     = ([x;1] @ [Wv;bv])    (tokens on partitions)
  per query-tile of 256 (ACT-exp paces at ~1.04us/key-ptile):
    per key-ptile kt2 (128 keys):
      S^T[:,h,:] = kT_h^T qT_h          (keys on partitions, 4 matmuls)
      P^T = exp(SCALE * S^T)            (one ACT op over all 4 heads)
      O[q,s,h]  += P^T_slice^T @ v_h    (64-col streams, queries on partitions)
      sums[q,h] += P^T_slice^T @ 1      (1-col streams, ~free)
    O /= sums (DVE per-partition scalars) -> transpose back (PE, identity)
    out[tokens] = sum_ct oT_ct^T @ Wo_ct  (chains of 2, K=128 each)
Host: out[b] = sum of the 4 group partials + b_o.
"""

import sys

sys.path.insert(0, "/opt/trn_rl_repo")

import numpy as np

B, N, D, H = 2, 2048, 1024, 16
SUB = D // H  # 64
GROUPS = 4  # tensor-parallel head groups
NH = H // GROUPS  # 4 local heads per core
CH = NH * SUB  # 256 local channels
NCORES = 8
QB = 256  # query tile
NQT = N // QB  # 8
KT = D // 128  # contraction ptiles
TOKT = N // 128  # token/key ptiles
SCALE = SUB ** -0.5


def build_nc(name="mha"):
    import concourse.mybir as mybir
    from concourse import bacc
    from concourse.tile import TileContext

    f32 = mybir.dt.float32
    bf16 = mybir.dt.bfloat16
    Exp = mybir.ActivationFunctionType.Exp

    nc = bacc.Bacc(None, name=name)
    # host-packed, partition-major layouts (see make_in_maps)
    xh = nc.dram_tensor("xh", [128, KT, N], bf16, kind="ExternalInput")
    wq = nc.dram_tensor("wq", [128, KT, CH], bf16, kind="ExternalInput")
    wk = nc.dram_tensor("wk", [128, KT, CH], bf16, kind="ExternalInput")
    wv = nc.dram_tensor("wv", [128, KT, CH], bf16, kind="ExternalInput")
    wvb = nc.dram_tensor("wvb", [1, CH], bf16, kind="ExternalInput")
    wo = nc.dram_tensor("wo", [128, 2, D], bf16, kind="ExternalInput")
    bqk = nc.dram_tensor("bqk", [128, 4], f32, kind="ExternalInput")
    ones_d = nc.dram_tensor("ones", [128, 128], bf16, kind="ExternalInput")
    ident_d = nc.dram_tensor("ident", [128, 128], bf16, kind="ExternalInput")
    out = nc.dram_tensor("out", [N, D], bf16, kind="ExternalOutput")

    with TileContext(nc) as tc:
        with tc.tile_pool(name="persist", bufs=1) as pp:
            xt = pp.tile([128, KT, N], bf16)
            wq_sb = pp.tile([128, KT, CH], bf16)
            wk_sb = pp.tile([128, KT, CH], bf16)
            wv_sb = pp.tile([128, KT, CH], bf16)
            wvb_sb = pp.tile([1, CH], bf16)
            wo_sb = pp.tile([128, 2, D], bf16)
            qT_sb = pp.tile([128, 2, N], bf16)
            kT_sb = pp.tile([128, 2, N], bf16)
            v_sb = pp.tile([128, TOKT, CH], bf16)
            oT_sb = pp.tile([128, 2, N], bf16)
            bqk_sb = pp.tile([128, 4], f32)
            ones_sb = pp.tile([128, 128], bf16)
            ident_sb = pp.tile([128, 128], bf16)

            # DMA issue order: earliest-needed first. x comes in 4 token
            # quarters so the first projections can start at ~4.5us.
            nc.sync.dma_start(wk_sb[:], wk[:])
            nc.sync.dma_start(xt[:, :, 0:256], xh[:, :, 0:256])
            nc.sync.dma_start(bqk_sb[:], bqk[:])
            nc.sync.dma_start(wq_sb[:], wq[:])
            nc.sync.dma_start(wv_sb[:], wv[:])
            nc.sync.dma_start(wvb_sb[:], wvb[:])
            nc.sync.dma_start(ones_sb[:], ones_d[:])
            nc.sync.dma_start(xt[:, :, 256:512], xh[:, :, 256:512])
            nc.sync.dma_start(ident_sb[:], ident_d[:])
            nc.sync.dma_start(xt[:, :, 512:768], xh[:, :, 512:768])
            nc.sync.dma_start(xt[:, :, 768:1024], xh[:, :, 768:1024])
            nc.sync.dma_start(wo_sb[:], wo[:])
            for sl in range(4, 8):
                nc.sync.dma_start(
                    xt[:, :, sl * 256 : (sl + 1) * 256],
                    xh[:, :, sl * 256 : (sl + 1) * 256],
                )

            with tc.tile_pool(name="stp", bufs=2, space="PSUM") as stp, \
                 tc.tile_pool(name="op_", bufs=1, space="PSUM") as op_, \
                 tc.tile_pool(name="smp", bufs=1, space="PSUM") as smp, \
                 tc.tile_pool(name="aux", bufs=2, space="PSUM") as aux, \
                 tc.tile_pool(name="ptp", bufs=22) as ptp, \
                 tc.tile_pool(name="osb", bufs=3) as osb, \
                 tc.tile_pool(name="rcpp", bufs=2) as rcpp, \
                 tc.tile_pool(name="stg", bufs=2) as stgp:

                def qk_proj(dst, wt, bcol, mt, s):
                    """dst[:, mt, 256s:+256] = (W^T x^T)[128ch, 256tok] + bias."""
                    ps = aux.tile([128, 512], f32, name="ps", tag="aux")
                    for kt in range(KT):
                        nc.tensor.matmul(
                            ps[:, 0:QB],
                            lhsT=wt[:, kt, mt * 128 : (mt + 1) * 128],
                            rhs=xt[:, kt, s * QB : (s + 1) * QB],
                            start=(kt == 0),
                            stop=(kt == KT - 1),
                        )
                    nc.vector.tensor_scalar_add(
                        dst[:, mt, s * QB : (s + 1) * QB],
                        ps[:, 0:QB],
                        bqk_sb[:, bcol + mt : bcol + mt + 1],
                    )

                def v_proj(tt):
                    """v_sb[:, tt, :] = ([x;1] @ [Wv;bv])[128tok, 256ch]."""
                    ps = aux.tile([128, 512], f32, name="psv", tag="aux")
                    for kt in range(KT):
                        nc.tensor.matmul(
                            ps[:, 0:CH],
                            lhsT=xt[:, kt, tt * 128 : (tt + 1) * 128],
                            rhs=wv_sb[:, kt, :],
                            start=(kt == 0),
                            stop=False,
                        )
                    nc.tensor.matmul(
                        ps[:, 0:CH],
                        lhsT=ones_sb[0:1, :],
                        rhs=wvb_sb[:],
                        start=False,
                        stop=True,
                    )
                    nc.vector.tensor_copy(v_sb[:, tt, :], ps[:, 0:CH])

                def transpose_pair(osb_t, qt, s):
                    """oT_sb[:, blk, qt*256+128s:+128] = osb_t[:, blk, :]^T."""
                    tr = aux.tile([128, 2, 128], bf16, name="tr", tag="aux")
                    for blk in range(2):
                        nc.tensor.transpose(
                            tr[:, blk, :], osb_t[:, blk, :], ident_sb[:]
                        )
                    for blk in range(2):
                        nc.vector.tensor_copy(
                            oT_sb[:, blk, qt * QB + 128 * s : qt * QB + 128 * (s + 1)],
                            tr[:, blk, :],
                        )

                def outproj(tt):
                    """out[tt*128:+128, :] = sum_ct oT_ct^T @ Wo_ct."""
                    stg = stgp.tile([128, D], bf16, name="stg", tag="stg")
                    for nt in range(2):
                        ps = aux.tile([128, 512], f32, name="ops", tag="aux")
                        for ct in range(2):
                            nc.tensor.matmul(
                                ps[:],
                                lhsT=oT_sb[:, ct, tt * 128 : (tt + 1) * 128],
                                rhs=wo_sb[:, ct, nt * 512 : (nt + 1) * 512],
                                start=(ct == 0),
                                stop=(ct == 1),
                            )
                        nc.vector.tensor_copy(stg[:, nt * 512 : (nt + 1) * 512], ps[:])
                    nc.sync.dma_start(out[tt * 128 : (tt + 1) * 128, :], stg[:])

                def emit(item):
                    kind = item[0]
                    if kind == "q":
                        qk_proj(qT_sb, wq_sb, 0, item[1], item[2])
                    elif kind == "k":
                        qk_proj(kT_sb, wk_sb, 2, item[1], item[2])
                    elif kind == "v":
                        v_proj(item[1])
                    elif kind == "tr":
                        transpose_pair(item[1], item[2], item[3])
                    elif kind == "o":
                        outproj(item[1])

                from collections import deque

                pending = deque()

                # PE p-state warmup: keep the PE continuously busy from t~0 so
                # it reaches full clock (3us ramp) before the real work lands.
                warm = pp.tile([128, 256], bf16)
                nc.vector.memset(warm[:], 0.0)
                wps = aux.tile([128, 512], f32, name="wps", tag="aux")
                for i in range(14):
                    nc.tensor.matmul(
                        wps[:, 0:256],
                        lhsT=warm[:, 0:128],
                        rhs=warm[:, :],
                        start=True,
                        stop=True,
                        skip_group_check=True,
                    )

                # front: projections for tokens 0-255 (x slice 0 only);
                # v goes after the first S/exp (emitted below) so the late
                # wv DMA never blocks the ACT stream start
                for mt in range(2):
                    qk_proj(kT_sb, wk_sb, 2, mt, 0)
                for mt in range(2):
                    qk_proj(qT_sb, wq_sb, 0, mt, 0)

                def s_exp(qt, kt2):
                    """S^T + exp for (qt, kt2); returns the pt tile."""
                    st = stp.tile([128, NH, QB], f32, name="st", tag="st")
                    for h in range(NH):
                        a = h % 2
                        nc.tensor.matmul(
                            st[:, h, :],
                            lhsT=kT_sb[
                                64 * a : 64 * a + 64,
                                h // 2,
                                kt2 * 128 : (kt2 + 1) * 128,
                            ],
                            rhs=qT_sb[
                                64 * a : 64 * a + 64,
                                h // 2,
                                qt * QB : (qt + 1) * QB,
                            ],
                            start=(h % 2 == 0),  # bank opener per 2 heads
                            stop=True,
                            skip_group_check=True,
                        )
                    pt = ptp.tile([128, NH, QB], bf16, name="pt", tag="pt")
                    nc.scalar.activation(pt[:], st[:], Exp, scale=SCALE)
                    return pt

                def pv(pt, kt2, o_ps, sm_ps):
                    first, last = kt2 == 0, kt2 == TOKT - 1
                    for s in range(2):
                        for h in range(NH):
                            nc.tensor.matmul(
                                o_ps[:, s, h, :],
                                lhsT=pt[:, h, 128 * s : 128 * (s + 1)],
                                rhs=v_sb[:, kt2, 64 * h : 64 * h + 64],
                                start=(first and s == 0 and h == 0),
                                stop=last,
                                skip_group_check=True,
                            )
                            nc.tensor.matmul(
                                sm_ps[:, 4 * s + h : 4 * s + h + 1],
                                lhsT=pt[:, h, 128 * s : 128 * (s + 1)],
                                rhs=ones_sb[:, 0:1],
                                start=(first and s == 0 and h == 0),
                                stop=last,
                                skip_group_check=True,
                            )

                def norm(qt, o_ps, sm_ps):
                    rcp = rcpp.tile([128, 8], f32, name="rcp", tag="rcp")
                    for s in range(2):
                        nc.vector.reciprocal(
                            rcp[:, 4 * s : 4 * s + 4], sm_ps[:, 4 * s : 4 * s + 4]
                        )
                        osb_t = osb.tile([128, 2, 128], bf16, name="osb", tag="osb")
                        for h in range(NH):
                            nc.vector.tensor_scalar_mul(
                                osb_t[:, h // 2, 64 * (h % 2) : 64 * (h % 2) + 64],
                                o_ps[:, s, h, :],
                                rcp[:, 4 * s + h : 4 * s + h + 1],
                            )
                        # post-qt work runs during later ACT windows
                        pending.append(("tr", osb_t, qt, s))
                        pending.append(("o", 2 * qt + s))

                # Deferred PV/normalize stream: ACT runs ahead of PV by up to
                # LAG tiles (deep pt pool), so the heavy qt0 projection
                # backlog never stalls the exp stream; the lag drains over
                # the last two qts to keep the tail short.
                LAG0 = 16
                DRAIN = (NQT - 2) * TOKT
                pv_q = deque()
                pt_next = s_exp(0, 0)
                v_proj(0)
                v_proj(1)
                for qt in range(NQT):
                    o_ps = op_.tile([128, 2, NH, SUB], f32, name="o_ps", tag="o")
                    sm_ps = smp.tile([128, 512], f32, name="sm_ps", tag="sm")
                    for kt2 in range(TOKT):
                        body = qt * TOKT + kt2
                        last = kt2 == TOKT - 1
                        pv_q.append(
                            lambda pt=pt_next, kt2=kt2, o=o_ps, sm=sm_ps: pv(
                                pt, kt2, o, sm
                            )
                        )
                        # software pipeline: S/exp of the NEXT tile go ahead of
                        # this tile's PV so the ACT stream never waits on PE
                        if not last:
                            pt_next = s_exp(qt, kt2 + 1)
                        elif qt + 1 < NQT:
                            pt_next = s_exp(qt + 1, 0)
                        # high-priority streamed projections
                        if qt == 0:
                            # all of kT and v must materialize inside qt0
                            s_next = kt2 // 2 + 1
                            if kt2 % 2 == 0 and s_next < 8:
                                qk_proj(kT_sb, wk_sb, 2, 0, s_next)
                                qk_proj(kT_sb, wk_sb, 2, 1, s_next)
                            if kt2 + 2 < TOKT:
                                v_proj(kt2 + 2)
                            elif kt2 == TOKT - 2:
                                v_proj(TOKT - 1)
                        if kt2 == 12 and qt + 1 < NQT:
                            qk_proj(qT_sb, wq_sb, 0, 0, qt + 1)
                        if kt2 == 13 and qt + 1 < NQT:
                            qk_proj(qT_sb, wq_sb, 0, 1, qt + 1)
                        if kt2 >= 2 and pending:
                            emit(pending.popleft())
                        lag = max(2, LAG0 - max(0, body - DRAIN))
                        while len(pv_q) > lag:
                            pv_q.popleft()()
                    pv_q.append(lambda qt=qt, o=o_ps, sm=sm_ps: norm(qt, o, sm))
                while pv_q:
                    pv_q.popleft()()
                while pending:
                    emit(pending.popleft())
    nc.finalize()
    return nc


def make_in_maps(x, W_qkv, b_qkv, W_o):
    """Shard full inputs into per-core input maps (core c: batch c//4, group c%4)."""
    import ml_dtypes

    bf16 = ml_dtypes.bfloat16
    x = np.asarray(x, dtype=np.float32)
    W_qkv = np.asarray(W_qkv, dtype=np.float32)
    b_qkv = np.asarray(b_qkv, dtype=np.float32)
    W_o = np.asarray(W_o, dtype=np.float32)

    def pack_w(w):  # [1024, CH] -> [128, KT, CH] partition-major
        return np.ascontiguousarray(
            w.reshape(KT, 128, CH).transpose(1, 0, 2).astype(bf16)
        )

    in_maps = []
    for c in range(NCORES):
        b, g = divmod(c, GROUPS)
        cols = slice(CH * g, CH * (g + 1))
        bq = b_qkv[0 * D : 1 * D][cols]
        bk = b_qkv[1 * D : 2 * D][cols]
        bqk = np.stack(
            [bq[0:128], bq[128:256], bk[0:128], bk[128:256]], axis=1
        ).astype(np.float32)
        xh = (
            x[b].T.reshape(KT, 128, N).transpose(1, 0, 2).astype(bf16)
        )  # [128, KT, N]
        m = {
            "xh": np.ascontiguousarray(xh),
            "wq": pack_w(W_qkv[:, 0 * D : 1 * D][:, cols]),
            "wk": pack_w(W_qkv[:, 1 * D : 2 * D][:, cols]),
            "wv": pack_w(W_qkv[:, 2 * D : 3 * D][:, cols]),
            "wvb": np.ascontiguousarray(
                b_qkv[2 * D : 3 * D][cols][None, :].astype(bf16)
            ),
            "wo": np.ascontiguousarray(
                W_o[cols, :].reshape(2, 128, D).transpose(1, 0, 2).astype(bf16)
            ),
            "bqk": np.ascontiguousarray(bqk),
            "ones": np.ones((128, 128), dtype=bf16),
            "ident": np.eye(128, dtype=bf16),
        }
        in_maps.append(m)
    return in_maps


_NC = None


def get_nc():
    global _NC
    if _NC is None:
        _NC = build_nc()
    return _NC


def kernel(x, W_qkv, b_qkv, W_o, b_o):
    from concourse import bass_utils

    b_o = np.asarray(b_o, dtype=np.float32)
    in_maps = make_in_maps(x, W_qkv, b_qkv, W_o)
    res = bass_utils.run_bass_kernel_spmd(get_nc(), in_maps, core_ids=list(range(NCORES)))
    out = np.empty((B, N, D), dtype=np.float32)
    for b in range(B):
        acc = res.results[4 * b]["out"].astype(np.float32)
        for g in range(1, GROUPS):
            acc += res.results[4 * b + g]["out"].astype(np.float32)
        out[b] = acc + b_o
    return out
